# revision 1
# baseline (speedup 1.0000x reference)
"""GAT (2-layer, heads=1) + pooling + MLP on 8 Trainium2 NeuronCores.

Strategy:
- Nodes are mapped to per-graph padded slots (SLOT = align128(max graph size)),
  graphs are sharded 16-per-core, and within each core slots are striped so
  that dst-block i holds slots s with s % NB == i.  Every 128-slot block then
  contains exactly 8 slots of each of the core's 16 graphs (partition p ->
  local graph p//8), which makes pooling segment boundaries compile-time.
- Edge aggregation: per-edge rows [h|s] are fetched with dma_gather (int16
  indices -> 4 src buckets); attention weights w = exp(leakyrelu(s_src +
  d_dst)) are computed on-chip (d expanded per edge via a one-hot *
  broadcast-d reduce); the segment softmax + feature sum is one matmul per
  128-edge tile accumulating [sum(w*h) | sum(w)] into a per-block PSUM.
- Self-loop edges are applied densely at finalize (no gather).
- Node tables ([h|s] rows, bf16) are built sharded and AllGathered; pooled
  [G,2H] is assembled with a single small AllGather; the final MLP is
  replicated.
"""
import sys

sys.path.insert(0, "/opt/trn_rl_repo")

import numpy as np
import ml_dtypes

import concourse.bacc as bacc
import concourse.bass as bass
import concourse.mybir as mybir
import concourse.tile as tile
from concourse import bass_utils
from concourse.masks import make_identity

bf16 = ml_dtypes.bfloat16
F32 = mybir.dt.float32
BF = mybir.dt.bfloat16
I16 = mybir.dt.int16
AL = mybir.AluOpType
ACT = mybir.ActivationFunctionType

NCORES = 8
G = 128
N = 100000
FIN = 64
H = 64
NEG = 0.2
NBUCKET = 4
GPC = G // NCORES  # graphs per core = 16
NEG_BIG = -1.0e30


# ---------------------------------------------------------------- host prep
def _host_prep(inputs):
    x = np.asarray(inputs["x"], np.float32)
    ei = np.asarray(inputs["edge_index"]).astype(np.int64)
    bid = np.asarray(inputs["batch_ids"]).astype(np.int64)

    cnt = np.bincount(bid, minlength=G).astype(np.int64)
    SLOT = int(np.ceil(max(cnt.max(), 128) / 128) * 128)
    NSLOT = GPC * SLOT
    NB = NSLOT // 128
    assert NB % 16 == 0, NB
    NGRP = NB // 16
    NSLOT_G = NCORES * NSLOT
    BUCKET = NSLOT_G // NBUCKET
    assert BUCKET * NBUCKET == NSLOT_G and BUCKET <= 32768

    gstart = np.zeros(G + 1, np.int64)
    gstart[1:] = np.cumsum(cnt)
    rank = np.arange(N, dtype=np.int64) - gstart[bid]
    slot_of = bid * SLOT + rank  # graph-padded slot, 0..NSLOT_G

    def pi(s):
        c, sl = s // NSLOT, s % NSLOT
        return c * NSLOT, (sl % NB) * 128 + sl // NB

    core_base, loc = pi(slot_of)
    pi_of = core_base + loc  # global pi row of each node

    # permuted x per core (pad rows zero)
    x_pi = np.zeros((NCORES, NSLOT, FIN), np.float32)
    x_pi[pi_of // NSLOT, pi_of % NSLOT] = x

    # masks / counts per core
    # local slot (block i, partition p) <-> graph-slot p*NB + i
    pp, ii = np.meshgrid(np.arange(128), np.arange(NB), indexing="ij")
    gslot = pp * NB + ii  # [128, NB] graph-padded local slot
    within = gslot % SLOT  # rank within graph
    lg = gslot // SLOT  # local graph 0..15 (== pp//8)
    mask01 = np.zeros((NCORES, 128, NB), np.float32)
    maskneg = np.zeros((NCORES, 128, NB), np.float32)
    for c in range(NCORES):
        real = within < cnt[c * GPC + lg]
        mask01[c] = real.astype(np.float32)
        maskneg[c] = np.where(real, 0.0, NEG_BIG).astype(np.float32)

    # edges (no self loops in the gather path)
    src, dst = ei[0], ei[1]
    ps = pi_of[src]
    pd = pi_of[dst]
    core = pd // NSLOT
    blk = (pd % NSLOT) // 128
    bkt = ps // BUCKET
    grp = blk // 16

    cnts = np.zeros((NCORES, NB, NBUCKET), np.int64)
    np.add.at(cnts, (core, blk, bkt), 1)
    # uniform tiles-per-block within each (group, bucket)
    tpb = np.zeros((NGRP, NBUCKET), np.int64)  # tiles per block
    for g in range(NGRP):
        for b in range(NBUCKET):
            m = cnts[:, 16 * g:16 * g + 16, b].max()
            tpb[g, b] = max(1, int(np.ceil(m / 128)))
    seg_tiles = (16 * tpb).astype(np.int64)  # tiles per (g,b) segment
    TT = int(seg_tiles.sum())  # total tiles per core per layer
    TOTSLOT = TT * 128

    # slot offsets: order (g, b, block-within-group, slot)
    seg_off = np.zeros((NGRP, NBUCKET), np.int64)
    acc = 0
    for g in range(NGRP):
        for b in range(NBUCKET):
            seg_off[g, b] = acc
            acc += seg_tiles[g, b] * 128

    order = np.lexsort((bkt, blk))  # edges sorted by (blk, bkt); core split below
    src_local = np.zeros((NCORES, 128, TOTSLOT // 128), np.int16)
    dst_loc = np.full((NCORES, 128, TOTSLOT // 128), -1.0, np.float32)
    for c in range(NCORES):
        sel = order[core[order] == c]
        sblk, sbkt = blk[sel], bkt[sel]
        sps, spd = ps[sel], pd[sel]
        # slot index for each edge: within its (g,b,block) run
        # run start: seg_off[g,b] + (blk%16)*tpb[g,b]*128; position = rank in run
        key = sblk * NBUCKET + sbkt
        # stable order already (blk, bkt); rank within run:
        runstart_mark = np.r_[True, key[1:] != key[:-1]]
        runid = np.cumsum(runstart_mark) - 1
        nruns = int(runid[-1]) + 1 if len(runid) else 0
        first = np.full(nruns, len(sel), np.int64)
        np.minimum.at(first, runid, np.arange(len(sel)))
        pos = np.arange(len(sel)) - first[runid]
        gg = sblk // 16
        slot = (seg_off[gg, sbkt] + (sblk % 16) * tpb[gg, sbkt] * 128 + pos)
        assert (pos < tpb[gg, sbkt] * 128).all()
        p_ = slot % 128
        t_ = slot // 128
        src_local[c, p_, t_] = (sps - sbkt * BUCKET).astype(np.int16)
        dst_loc[c, p_, t_] = (spd % NSLOT % 128).astype(np.float32)

    # wrapped int16 gather indices per (g,b) call, concatenated along cols
    idx_w = np.zeros((NCORES, 128, TOTSLOT // 16), np.int16)
    for c in range(NCORES):
        flat = np.zeros(TOTSLOT, np.int16)
        sl = src_local[c]
        flat[np.arange(TOTSLOT)] = sl[np.arange(TOTSLOT) % 128,
                                      np.arange(TOTSLOT) // 128]
        w = flat.reshape(TOTSLOT // 16, 16).T  # [16, TOTSLOT//16]
        idx_w[c] = np.tile(w, (8, 1))

    # weights
    W1 = np.asarray(inputs["W1"], np.float32)
    W2 = np.asarray(inputs["W2"], np.float32)
    waug1 = np.concatenate(
        [W1, (W1 @ np.asarray(inputs["a_src1"], np.float32))[:, None],
         (W1 @ np.asarray(inputs["a_dst1"], np.float32))[:, None]], axis=1)
    waug2 = np.concatenate(
        [W2, (W2 @ np.asarray(inputs["a_src2"], np.float32))[:, None],
         (W2 @ np.asarray(inputs["a_dst2"], np.float32))[:, None]], axis=1)

    b1 = np.asarray(inputs["b1"], np.float32)
    b2v = np.asarray(inputs["b2"], np.float32)
    lin1_W = np.asarray(inputs["lin1_W"], np.float32)
    lin1_b = np.asarray(inputs["lin1_b"], np.float32)
    lin2_W = np.asarray(inputs["lin2_W"], np.float32)
    lin2_b = np.asarray(inputs["lin2_b"], np.float32)

    npadneg = np.zeros((NCORES, 64, GPC), np.float32)
    invcnt = np.zeros((NCORES, 64, GPC), np.float32)
    for c in range(NCORES):
        npadneg[c] = -np.broadcast_to(
            (SLOT - cnt[c * GPC:(c + 1) * GPC]).astype(np.float32), (64, GPC))
        invcnt[c] = np.broadcast_to(
            1.0 / np.maximum(cnt[c * GPC:(c + 1) * GPC], 1.0), (64, GPC))

    tpl = dict(SLOT=SLOT, NSLOT=NSLOT, NB=NB, NGRP=NGRP, NSLOT_G=NSLOT_G,
               BUCKET=BUCKET, tpb=tpb, seg_tiles=seg_tiles, seg_off=seg_off,
               TT=TT)

    per_core = []
    for c in range(NCORES):
        per_core.append({
            "xT_own": np.ascontiguousarray(x_pi[c].T),
            "idx_w": idx_w[c],
            "dst_loc": dst_loc[c],
            "mask01": mask01[c],
            "maskpad": 1.0 - mask01[c],
            "maskneg": maskneg[c],
            "npadneg": npadneg[c],
            "invcnt": invcnt[c],
            "waug1": waug1,
            "waug2_bf": waug2.astype(bf16),
            "b1_tile": np.broadcast_to(b1, (128, 64)).copy(),
            "b1_tile2": np.broadcast_to(np.asarray(inputs["b2"], np.float32),
                                        (128, 64)).copy(),
            "lin1_W": lin1_W,
            "lin1b_tile": np.broadcast_to(lin1_b, (128, 64)).copy(),
            "lin2row": np.broadcast_to(lin2_W[:, 0], (128, 64)).copy(),
            "b2col": np.full((128, 1), lin2_b[0], np.float32),
            "rb_cols": np.stack([np.maximum(b1, 0.0),
                                 np.maximum(b2v, 0.0)], axis=1),
        })
    return tpl, per_core


# ---------------------------------------------------------------- device bld
DEBUG = False


def _build(tpl):
    NSLOT = tpl["NSLOT"]
    NB = tpl["NB"]
    NGRP = tpl["NGRP"]
    NSLOT_G = tpl["NSLOT_G"]
    BUCKET = tpl["BUCKET"]
    tpb = tpl["tpb"]
    seg_tiles = tpl["seg_tiles"]
    TT = tpl["TT"]

    nc = bacc.Bacc("TRN2", target_bir_lowering=False, debug=False,
                   num_devices=NCORES)

    # inputs
    xT_own = nc.dram_tensor("xT_own", [FIN, NSLOT], F32,
                            kind="ExternalInput")
    idx_w = nc.dram_tensor("idx_w", [128, TT * 8], I16, kind="ExternalInput")
    dst_loc = nc.dram_tensor("dst_loc", [128, TT], F32, kind="ExternalInput")
    mask01 = nc.dram_tensor("mask01", [128, NB], F32, kind="ExternalInput")
    maskpad = nc.dram_tensor("maskpad", [128, NB], F32, kind="ExternalInput")
    npadneg = nc.dram_tensor("npadneg", [64, GPC], F32, kind="ExternalInput")
    invcnt = nc.dram_tensor("invcnt", [64, GPC], F32, kind="ExternalInput")
    waug1 = nc.dram_tensor("waug1", [64, 66], F32, kind="ExternalInput")
    waug2_bf = nc.dram_tensor("waug2_bf", [64, 66], BF, kind="ExternalInput")
    b1_tile = nc.dram_tensor("b1_tile", [128, 64], F32, kind="ExternalInput")
    b1_tile2 = nc.dram_tensor("b1_tile2", [128, 64], F32, kind="ExternalInput")
    lin1_W = nc.dram_tensor("lin1_W", [128, 64], F32, kind="ExternalInput")
    lin1b_tile = nc.dram_tensor("lin1b_tile", [128, 64], F32,
                                kind="ExternalInput")
    lin2row = nc.dram_tensor("lin2row", [128, 64], F32, kind="ExternalInput")
    b2col = nc.dram_tensor("b2col", [128, 1], F32, kind="ExternalInput")
    rb_cols = nc.dram_tensor("rb_cols", [64, 2], F32, kind="ExternalInput")

    out_final = nc.dram_tensor("out_final", [128, 1], F32,
                               kind="ExternalOutput")
    if DEBUG:
        dbg_tab = nc.dram_tensor("dbg_tab", [NSLOT, 65], BF,
                                 kind="ExternalOutput")
        dbg_d = nc.dram_tensor("dbg_d", [NB, 128], F32, kind="ExternalOutput")
        dbg_hT = nc.dram_tensor("dbg_hT", [64, NSLOT], BF,
                                kind="ExternalOutput")
        dbg_pool = nc.dram_tensor("dbg_pool", [64, 4 * GPC], F32,
                                  kind="ExternalOutput")
        dbg_z = nc.dram_tensor("dbg_z", [G, 128], F32, kind="ExternalOutput")

    with tile.TileContext(nc) as tc:
        with (
            tc.tile_pool(name="const", bufs=1) as cp,
            tc.tile_pool(name="stage", bufs=1) as stp,
            tc.tile_pool(name="dram", bufs=1, space="DRAM") as dr,
        ):
            # ---- constants in SBUF
            iota_i = cp.tile([128, 128], mybir.dt.int32)
            nc.gpsimd.iota(iota_i[:], pattern=[[1, 128]], base=0,
                           channel_multiplier=0)
            iota_bf = cp.tile([128, 128], BF)
            nc.vector.tensor_copy(iota_bf[:], iota_i[:])
            ident = cp.tile([128, 128], F32)
            make_identity(nc, ident[:])
            ones_row = cp.tile([1, 128], F32)
            nc.gpsimd.memset(ones_row[:], 1.0)
            ones_bf = cp.tile([128, 1], BF)
            nc.gpsimd.memset(ones_bf[:], 1.0)
            waug1_sb = cp.tile([64, 66], F32)
            nc.sync.dma_start(waug1_sb[:], waug1[:, :])
            waug2_sb = cp.tile([64, 66], BF)
            nc.sync.dma_start(waug2_sb[:], waug2_bf[:, :])
            b1t = cp.tile([128, 64], F32)
            nc.sync.dma_start(b1t[:], b1_tile[:, :])
            b2t = cp.tile([128, 64], F32)
            nc.sync.dma_start(b2t[:], b1_tile2[:, :])
            m01 = cp.tile([128, NB], F32)
            nc.sync.dma_start(m01[:], mask01[:, :])
            mpad = cp.tile([128, NB], F32)
            nc.sync.dma_start(mpad[:], maskpad[:, :])
            dl_all = cp.tile([128, TT], F32)
            nc.sync.dma_start(dl_all[:], dst_loc[:, :])

            # persistent staging
            h_T = stp.tile([64, NSLOT], BF, tag="h_T", name="h_T")  # layer1 out, pi order
            dstage = [stp.tile([128, NB], F32, tag=f"dstage{l}", name=f"dstage{l}")
                      for l in range(2)]
            pool_mx = [stp.tile([64, GPC], F32, tag=f"pmx{l}", name=f"pmx{l}")
                       for l in range(2)]
            pool_sm = [stp.tile([64, GPC], F32, tag=f"psm{l}", name=f"psm{l}")
                       for l in range(2)]
            for l in range(2):
                nc.vector.memset(pool_mx[l][:], NEG_BIG)
                nc.vector.memset(pool_sm[l][:], 0.0)

            # DRAM scratch
            table = [dr.tile([NSLOT_G, 128], BF, tag=f"tab{l}", name=f"tab{l}")
                     for l in range(2)]
            tab_own = [dr.tile([NSLOT, 128], BF, tag=f"tabown{l}", name=f"tabown{l}")
                       for l in range(2)]
            d2d = [dr.tile([NB, 128], F32, tag=f"d2d{l}", name=f"d2d{l}") for l in range(2)]
            pool_bounce_in = dr.tile([GPC, 128], F32)
            pool_bounce_out = dr.tile([G, 128], F32)

            # ===== table 1 build: replicated full + own shard for d =====
            with (
                tc.tile_pool(name="p1", bufs=3) as p1,
                tc.tile_pool(name="p1x", bufs=3) as p1x,
                tc.tile_pool(name="p1ps", bufs=2, space="PSUM") as p1ps,
            ):
                SLAB = 32
                for base in range(0, NB, SLAB):
                    ns = min(SLAB, NB - base)
                    xT = p1x.tile([64, SLAB * 128], F32, tag="xTs")
                    nc.sync.dma_start(
                        xT[:, 0:ns * 128],
                        xT_own[:, 128 * base:128 * (base + ns)])
                    rows = p1.tile([128, SLAB, 65], BF, tag="rows")
                    for j in range(ns):
                        hps = p1ps.tile([128, 66], F32, space="PSUM",
                                        tag="hps")
                        nc.tensor.matmul(
                            hps[:], lhsT=xT[:, 128 * j:128 * (j + 1)],
                            rhs=waug1_sb[:], start=True, stop=True)
                        nc.scalar.activation(rows[:, j, :], hps[:, 0:65],
                                             ACT.Copy)
                        nc.vector.tensor_copy(
                            dstage[0][:, base + j:base + j + 1],
                            hps[:, 65:66])
                    nc.sync.dma_start(
                        tab_own[0].rearrange("(s r) c -> r s c", r=128)[
                            :, base:base + ns, 0:65],
                        rows[:, 0:ns, :])
                # d transpose -> DRAM [NB, 128]
                dt_ps = p1ps.tile([NB, 128], F32, space="PSUM", tag="dt")
                nc.tensor.transpose(dt_ps[:], dstage[0][:], ident[:])
                dt_sb = p1.tile([NB, 128], F32, tag="dts")
                nc.vector.tensor_copy(dt_sb[:], dt_ps[:])
                nc.sync.dma_start(d2d[0][:, :], dt_sb[:])

            nc.gpsimd.collective_compute(
                "AllGather", AL.bypass,
                replica_groups=[list(range(NCORES))],
                ins=[tab_own[0].opt()], outs=[table[0].opt()])

            # ================= edge pass (both layers) =================
            def edge_pass(l):
                tab = table[l]
                with (
                    tc.tile_pool(name=f"eg{l}", bufs=2) as eg,
                    tc.tile_pool(name=f"eq{l}", bufs=2) as eqp,
                    tc.tile_pool(name=f"ep{l}", bufs=1, space="PSUM") as eps,
                    tc.tile_pool(name=f"et{l}", bufs=2, space="PSUM") as ept,
                    tc.tile_pool(name=f"ef{l}", bufs=3) as ef,
                ):
                    for g in range(NGRP):
                        # broadcast d rows of the group's 16 blocks
                        dbc = ef.tile([128, 16, 128], F32, tag="dbc")
                        nc.sync.dma_start(
                            dbc[:],
                            d2d[l][16 * g:16 * (g + 1), :].rearrange(
                                "(o r) c -> o r c", o=1).to_broadcast(
                                [128, 16, 128]))
                        dbc_bf = ef.tile([128, 16, 128], BF, tag="dbcb")
                        nc.vector.tensor_copy(dbc_bf[:], dbc[:])

                        psums = [eps.tile([128, 260], F32, space="PSUM",
                                          tag=f"ps{k}", name=f"ps{k}")
                                 for k in range(4)]
                        for ps_ in psums:
                            nc.vector.memset(ps_[:], 0.0)

                        for b in range(NBUCKET):
                            ntile = int(seg_tiles[g, b])
                            Tb = int(tpb[g, b])
                            off = int(tpl["seg_off"][g, b]) // 128  # tile off
                            # gather the whole (g,b) segment
                            idx_sb = eg.tile([128, ntile * 8], I16, tag="idx")
                            nc.sync.dma_start(
                                idx_sb[:],
                                idx_w[:, off * 8:(off + ntile) * 8])
                            ch = eg.tile([128, ntile, 128], BF, tag="ch")
                            nc.gpsimd.dma_gather(
                                out_ap=ch[:],
                                in_ap=tab[b * BUCKET:(b + 1) * BUCKET, :],
                                idxs_ap=idx_sb[:],
                                num_idxs=ntile * 128,
                                num_idxs_reg=ntile * 128,
                                elem_size=128,
                                single_packet=False)
                            dl = dl_all[:, off:off + ntile]
                            # per tile: lhsT = (iota==dstloc)*|d|  (the |d|
                            # row-scale cancels in the softmax division);
                            # accum of (iota==dstloc)*d gives d_edge.
                            lhsT = eqp.tile([128, ntile, 128], BF, tag="eq")
                            dedge = ef.tile([128, ntile], F32, tag="dedge")
                            for tt in range(ntile):
                                k16 = tt // Tb
                                nc.vector.scalar_tensor_tensor(
                                    out=lhsT[:, tt, :], in0=iota_bf[:],
                                    scalar=dl[:, tt:tt + 1],
                                    in1=dbc_bf[:, k16, :],
                                    op0=AL.is_equal, op1=AL.mult,
                                    accum_out=dedge[:, tt:tt + 1])
                            # z = s + d ; w = exp(max(0.2 z, z))
                            z = ef.tile([128, ntile], F32, tag="z")
                            nc.vector.tensor_tensor(
                                out=z[:], in0=ch[:, :, 64], in1=dedge[:],
                                op=AL.add)
                            nc.vector.scalar_tensor_tensor(
                                out=z[:], in0=z[:], scalar=NEG, in1=z[:],
                                op0=AL.mult, op1=AL.max)
                            w = ef.tile([128, ntile], F32, tag="w")
                            nc.scalar.activation(w[:], z[:], ACT.Exp)
                            # ones into s slot for the denominator column
                            nc.vector.tensor_copy(
                                ch[:, :, 64],
                                ones_bf[:].to_broadcast([128, ntile]))
                            # rhs2 = ch[:, :, 0:65] * w (bulk)
                            ch2 = eqp.tile([128, ntile, 65], BF, tag="dmul",
                                           name="ch2")
                            nc.vector.tensor_tensor(
                                out=ch2[:], in0=ch[:, :, 0:65],
                                in1=w[:].to_broadcast([128, ntile, 65]),
                                op=AL.mult)
                            # matmuls
                            for i in range(16):
                                ps = psums[i // 4]
                                csl = slice(65 * (i % 4), 65 * (i % 4) + 65)
                                for t in range(Tb):
                                    tt = i * Tb + t
                                    nc.tensor.matmul(
                                        ps[:, csl],
                                        lhsT=lhsT[:, tt, :],
                                        rhs=ch2[:, tt, :],
                                        start=False,
                                        stop=(b == NBUCKET - 1 and t == Tb - 1))

                        # ---- finalize the group's 16 blocks
                        for i in range(16):
                            blk_id = 16 * g + i
                            ps = psums[i // 4]
                            csl = slice(65 * (i % 4), 65 * (i % 4) + 65)
                            # self loops (dense): rows of own dst block
                            row = ef.tile([128, 65], BF, tag="slrow")
                            nc.sync.dma_start(
                                row[:], tab_own[l][128 * blk_id:
                                                   128 * blk_id + 128, 0:65])
                            zs = ef.tile([128, 1], F32, tag="zs")
                            nc.vector.tensor_tensor(
                                out=zs[:], in0=row[:, 64:65],
                                in1=dstage[l][:, blk_id:blk_id + 1],
                                op=AL.add)
                            nc.vector.scalar_tensor_tensor(
                                out=zs[:], in0=zs[:], scalar=NEG, in1=zs[:],
                                op0=AL.mult, op1=AL.max)
                            ws = ef.tile([128, 1], F32, tag="ws")
                            nc.scalar.activation(ws[:], zs[:], ACT.Exp)
                            nc.vector.tensor_tensor(
                                out=ws[:], in0=ws[:],
                                in1=m01[:, blk_id:blk_id + 1], op=AL.mult)
                            nc.vector.tensor_tensor(
                                out=ws[:], in0=ws[:],
                                in1=dstage[l][:, blk_id:blk_id + 1],
                                op=AL.mult)
                            nc.vector.tensor_copy(
                                row[:, 64:65], ones_bf[:])
                            nc.vector.scalar_tensor_tensor(
                                out=ps[:, csl], in0=row[:, :],
                                scalar=ws[:], in1=ps[:, csl],
                                op0=AL.mult, op1=AL.add)
                            # divide + bias + relu
                            den = ef.tile([128, 1], F32, tag="den")
                            nc.vector.tensor_tensor(
                                out=den[:],
                                in0=ps[:, csl.start + 64:csl.start + 65],
                                in1=mpad[:, blk_id:blk_id + 1], op=AL.add)
                            rec = ef.tile([128, 1], F32, tag="rec")
                            nc.vector.reciprocal(rec[:], den[:])
                            hmid = ef.tile([128, 64], F32, tag="hmid")
                            nc.vector.scalar_tensor_tensor(
                                out=hmid[:],
                                in0=ps[:, csl.start:csl.start + 64],
                                scalar=rec[:], in1=b1t[:] if l == 0 else b2t[:],
                                op0=AL.mult, op1=AL.add)
                            hout = ef.tile([128, 64], F32, tag="hout")
                            nc.scalar.activation(hout[:], hmid[:], ACT.Relu)
                            # mask pads to exactly 0 (safe for max: h1 >= 0)
                            nc.vector.tensor_scalar_mul(
                                hout[:], hout[:], m01[:, blk_id:blk_id + 1])
                            # transpose (PE) -> [64, 128]
                            hT = ept.tile([64, 128], F32, space="PSUM",
                                          tag="hT")
                            nc.tensor.transpose(hT[:], hout[:], ident[:])
                            if l == 0:
                                nc.vector.tensor_copy(
                                    h_T[:, 128 * blk_id:128 * (blk_id + 1)],
                                    hT[:])
                            # sum pool
                            red = ef.tile([64, GPC], F32, tag="red")
                            nc.vector.tensor_reduce(
                                red[:],
                                hT[:].rearrange("f (g e) -> f g e", g=GPC),
                                axis=mybir.AxisListType.X, op=AL.add)
                            nc.vector.tensor_tensor(
                                out=pool_sm[l][:], in0=pool_sm[l][:],
                                in1=red[:], op=AL.add)
                            # max pool
                            redm = ef.tile([64, GPC], F32, tag="redm")
                            nc.vector.tensor_reduce(
                                redm[:],
                                hT[:].rearrange("f (g e) -> f g e", g=GPC),
                                axis=mybir.AxisListType.X, op=AL.max)
                            nc.vector.tensor_tensor(
                                out=pool_mx[l][:], in0=pool_mx[l][:],
                                in1=redm[:], op=AL.max)

            edge_pass(0)

            # ================= table 2 build =================
            with (
                tc.tile_pool(name="p3", bufs=3) as p3,
                tc.tile_pool(name="p3ps", bufs=2, space="PSUM") as p3ps,
            ):
                for i in range(NB):
                    hps = p3ps.tile([128, 66], F32, space="PSUM", tag="hps2")
                    nc.tensor.matmul(
                        hps[:], lhsT=h_T[:, 128 * i:128 * (i + 1)],
                        rhs=waug2_sb[:], start=True, stop=True)
                    row = p3.tile([128, 65], BF, tag="row2")
                    nc.vector.tensor_copy(row[:], hps[:, 0:65])
                    nc.sync.dma_start(
                        tab_own[1][128 * i:128 * (i + 1), 0:65], row[:])
                    nc.vector.tensor_copy(dstage[1][:, i:i + 1], hps[:, 65:66])
                dt_ps = p3ps.tile([NB, 128], F32, space="PSUM", tag="dt2")
                nc.tensor.transpose(dt_ps[:], dstage[1][:], ident[:])
                dt_sb = p3.tile([NB, 128], F32, tag="dts2")
                nc.vector.tensor_copy(dt_sb[:], dt_ps[:])
                nc.sync.dma_start(d2d[1][:, :], dt_sb[:])

            nc.gpsimd.collective_compute(
                "AllGather", AL.bypass,
                replica_groups=[list(range(NCORES))],
                ins=[tab_own[1].opt()], outs=[table[1].opt()])

            edge_pass(1)

            if DEBUG:
                nc.sync.dma_start(dbg_tab[:, :], tab_own[0][:, 0:65])
                dbg_d_sb = stp.tile([NB, 128], F32, name="dbgd")
                nc.sync.dma_start(dbg_d_sb[:], d2d[0][:, :])
                nc.sync.dma_start(dbg_d[:, :], dbg_d_sb[:])
                nc.sync.dma_start(dbg_hT[:, :], h_T[:])
                for li in range(2):
                    nc.sync.dma_start(
                        dbg_pool[:, li * GPC:(li + 1) * GPC], pool_mx[li][:])
                    nc.sync.dma_start(
                        dbg_pool[:, (2 + li) * GPC:(3 + li) * GPC],
                        pool_sm[li][:])

            # ================= pooling combine + MLP =================
            with (
                tc.tile_pool(name="p5", bufs=2) as p5,
                tc.tile_pool(name="p5ps", bufs=1, space="PSUM") as p5ps,
            ):
                icn = p5.tile([64, GPC], F32)
                nc.sync.dma_start(icn[:], invcnt[:, :])
                mxh = p5.tile([64, GPC], F32)
                nc.vector.tensor_tensor(out=mxh[:], in0=pool_mx[0][:],
                                        in1=pool_mx[1][:], op=AL.add)
                smh = p5.tile([64, GPC], F32)
                nc.vector.tensor_tensor(out=smh[:], in0=pool_sm[0][:],
                                        in1=pool_sm[1][:], op=AL.add)
                nc.vector.tensor_tensor(out=smh[:], in0=smh[:], in1=icn[:],
                                        op=AL.mult)
                # transpose to graph-major [GPC, 128] and AllGather
                zloc = p5.tile([GPC, 128], F32)
                mxT = p5ps.tile([GPC, 64], F32, space="PSUM", tag="mxT")
                nc.tensor.transpose(mxT[:], mxh[:], ident[0:64, 0:64])
                nc.vector.tensor_copy(zloc[:, 0:64], mxT[:])
                smT = p5ps.tile([GPC, 64], F32, space="PSUM", tag="smT")
                nc.tensor.transpose(smT[:], smh[:], ident[0:64, 0:64])
                nc.vector.tensor_copy(zloc[:, 64:128], smT[:])
                nc.sync.dma_start(pool_bounce_in[:, :], zloc[:])
                nc.gpsimd.collective_compute(
                    "AllGather", AL.bypass,
                    replica_groups=[list(range(NCORES))],
                    ins=[pool_bounce_in.opt()], outs=[pool_bounce_out.opt()])
                zg = p5.tile([G, 128], F32)
                nc.sync.dma_start(zg[:], pool_bounce_out[:, :])
                if DEBUG:
                    nc.sync.dma_start(dbg_z[:, :], zg[:])
                zT_ps = p5ps.tile([128, G], F32, space="PSUM", tag="zT")
                nc.tensor.transpose(zT_ps[:], zg[:], ident[:])
                zT = p5.tile([128, G], F32)
                nc.vector.tensor_copy(zT[:], zT_ps[:])
                l1w = p5.tile([128, 64], F32)
                nc.sync.dma_start(l1w[:], lin1_W[:, :])
                mlp_ps = p5ps.tile([G, 64], F32, space="PSUM", tag="mlp")
                nc.tensor.matmul(mlp_ps[:], lhsT=zT[:], rhs=l1w[:],
                                 start=True, stop=True)
                l1b = p5.tile([128, 64], F32)
                nc.sync.dma_start(l1b[:], lin1b_tile[:, :])
                z1 = p5.tile([G, 64], F32)
                nc.vector.tensor_tensor(out=z1[:], in0=mlp_ps[:], in1=l1b[:],
                                        op=AL.add)
                nc.scalar.activation(z1[:], z1[:], ACT.Relu)
                l2r = p5.tile([128, 64], F32)
                nc.sync.dma_start(l2r[:], lin2row[:, :])
                z2 = p5.tile([G, 64], F32)
                nc.vector.tensor_tensor(out=z2[:], in0=z1[:], in1=l2r[:],
                                        op=AL.mult)
                ored = p5.tile([G, 1], F32)
                nc.vector.tensor_reduce(ored[:], z2[:],
                                        axis=mybir.AxisListType.X, op=AL.add)
                b2c = p5.tile([128, 1], F32)
                nc.sync.dma_start(b2c[:], b2col[:, :])
                nc.vector.tensor_tensor(out=ored[:], in0=ored[:], in1=b2c[:],
                                        op=AL.add)
                nc.sync.dma_start(out_final[:, :], ored[:])

    nc.compile()
    return nc


# ---------------------------------------------------------------- entry
def kernel(**inputs) -> np.ndarray:
    tpl, per_core = _host_prep(inputs)
    nc = _build(tpl)
    in_maps = []
    for c in range(NCORES):
        pc = per_core[c]
        in_maps.append({
            "xT_own": pc["xT_own"],
            "idx_w": pc["idx_w"],
            "dst_loc": pc["dst_loc"],
            "mask01": pc["mask01"],
            "maskpad": pc["maskpad"],
            "npadneg": pc["npadneg"],
            "invcnt": pc["invcnt"],
            "waug1": pc["waug1"],
            "waug2_bf": pc["waug2_bf"],
            "b1_tile": pc["b1_tile"],
            "b1_tile2": pc["b1_tile2"],
            "lin1_W": pc["lin1_W"],
            "lin1b_tile": pc["lin1b_tile"],
            "lin2row": pc["lin2row"],
            "b2col": pc["b2col"],
            "rb_cols": pc["rb_cols"],
        })
    res = bass_utils.run_bass_kernel_spmd(
        nc, in_maps, core_ids=list(range(NCORES)))
    out = np.asarray(res.results[0]["out_final"]).reshape(G)
    return out.astype(np.float32)



# revision 24
# speedup vs baseline: 403.2076x; 403.2076x over previous
"""GAT (2-layer, heads=1) + pooling + MLP on 8 Trainium2 NeuronCores.

Strategy:
- Nodes are mapped to per-graph padded slots (SLOT = align128(max graph size)),
  graphs are sharded 16-per-core, and within each core slots are striped so
  that dst-block i holds slots s with s % NB == i.  Every 128-slot block then
  contains exactly 8 slots of each of the core's 16 graphs (partition p ->
  local graph p//8), which makes pooling segment boundaries compile-time.
- Edge aggregation: per-edge rows [h|s] are fetched with dma_gather (int16
  indices -> 4 src buckets).  The edge->dst one-hot matrices are STATIC
  (graph structure), so they are precomputed on host in fp8 and streamed
  from HBM: ohT tiles give d_dst per edge via a tiny PE matvec against the
  block's d column; oh tiles drive the segment-softmax scatter matmul
  (one [128e x 128dst] x [128e x 65] matmul per 128-edge tile accumulating
  [sum(w*h) | sum(w)] into a per-block PSUM).
- Self-loop edges are applied densely at finalize (no gather); finalize is
  batched per 16-block group, and the layer-2 node table is built in the
  same pass.  Pooling (max+sum) is done at the end with two strided
  tensor_reduce ops over the stored [64, NSLOT] feature tables.
- Node tables are built sharded and AllGathered; pooled [G,2H] is assembled
  with a single small AllGather; the final MLP is replicated.
"""
import sys

sys.path.insert(0, "/opt/trn_rl_repo")

import numpy as np
import ml_dtypes

import concourse.bacc as bacc
import concourse.bass as bass
import concourse.mybir as mybir
import concourse.tile as tile
from concourse import bass_utils
from concourse.masks import make_identity

bf16 = ml_dtypes.bfloat16
fp8 = ml_dtypes.float8_e4m3
F32 = mybir.dt.float32
BF = mybir.dt.bfloat16
F8 = mybir.dt.float8e4
I16 = mybir.dt.int16
AL = mybir.AluOpType
ACT = mybir.ActivationFunctionType

NCORES = 8
G = 128
N = 100000
FIN = 64
H = 64
NEG = 0.2
NBUCKET = 4
GPC = G // NCORES  # graphs per core = 16
NEG_BIG = -1.0e30


# ---------------------------------------------------------------- host prep
def _host_prep(inputs):
    x = np.asarray(inputs["x"], np.float32)
    ei = np.asarray(inputs["edge_index"]).astype(np.int64)
    bid = np.asarray(inputs["batch_ids"]).astype(np.int64)

    cnt = np.bincount(bid, minlength=G).astype(np.int64)
    SLOT = int(np.ceil(max(cnt.max(), 128) / 128) * 128)
    NSLOT = GPC * SLOT
    NB = NSLOT // 128
    assert NB % 16 == 0, NB
    NGRP = NB // 16
    NSLOT_G = NCORES * NSLOT
    BUCKET = NSLOT_G // NBUCKET
    assert BUCKET * NBUCKET == NSLOT_G and BUCKET <= 32768

    gstart = np.zeros(G + 1, np.int64)
    gstart[1:] = np.cumsum(cnt)
    rank = np.arange(N, dtype=np.int64) - gstart[bid]
    slot_of = bid * SLOT + rank  # graph-padded slot, 0..NSLOT_G

    def pi(s):
        c, sl = s // NSLOT, s % NSLOT
        return c * NSLOT, (sl % NB) * 128 + sl // NB

    core_base, loc = pi(slot_of)
    pi_of = core_base + loc  # global pi row of each node

    # permuted x per core (pad rows zero)
    x_pi = np.zeros((NCORES, NSLOT, FIN), np.float32)
    x_pi[pi_of // NSLOT, pi_of % NSLOT] = x

    # masks / counts per core
    # local slot (block i, partition p) <-> graph-slot p*NB + i
    pp, ii = np.meshgrid(np.arange(128), np.arange(NB), indexing="ij")
    gslot = pp * NB + ii  # [128, NB] graph-padded local slot
    within = gslot % SLOT  # rank within graph
    lg = gslot // SLOT  # local graph 0..15 (== pp//8)
    mask01 = np.zeros((NCORES, 128, NB), np.float32)
    for c in range(NCORES):
        real = within < cnt[c * GPC + lg]
        mask01[c] = real.astype(np.float32)

    # edges (no self loops in the gather path)
    # src-bucket layout: table rows ordered [quarter, core, loc%QUARTER] so
    # bucket q is complete as soon as the q-th quarter-AllGather lands.
    QUARTER = NSLOT // NBUCKET
    src, dst = ei[0], ei[1]
    ps = pi_of[src]
    pd = pi_of[dst]
    core = pd // NSLOT
    blk = (pd % NSLOT) // 128
    bkt = (ps % NSLOT) // QUARTER
    idx_in_bucket = (ps // NSLOT) * QUARTER + (ps % NSLOT) % QUARTER
    grp = blk // 16

    cnts = np.zeros((NCORES, NB, NBUCKET), np.int64)
    np.add.at(cnts, (core, blk, bkt), 1)
    # uniform tiles-per-block within each (group, bucket)
    tpb = np.zeros((NGRP, NBUCKET), np.int64)  # tiles per block
    for g in range(NGRP):
        for b in range(NBUCKET):
            m = cnts[:, 16 * g:16 * g + 16, b].max()
            tpb[g, b] = max(1, int(np.ceil(m / 128)))
    seg_tiles = (16 * tpb).astype(np.int64)  # tiles per (g,b) segment
    TT = int(seg_tiles.sum())  # total tiles per core per layer
    TOTSLOT = TT * 128

    # slot offsets: order (g, b, block-within-group, slot)
    seg_off = np.zeros((NGRP, NBUCKET), np.int64)
    acc = 0
    for g in range(NGRP):
        for b in range(NBUCKET):
            seg_off[g, b] = acc
            acc += seg_tiles[g, b] * 128

    order = np.lexsort((bkt, blk))  # edges sorted by (blk, bkt); core split below
    src_local = np.zeros((NCORES, 128, TOTSLOT // 128), np.int16)
    dst_loc = np.full((NCORES, 128, TOTSLOT // 128), -1, np.int64)
    for c in range(NCORES):
        sel = order[core[order] == c]
        sblk, sbkt = blk[sel], bkt[sel]
        sib, spd = idx_in_bucket[sel], pd[sel]
        # slot index for each edge: within its (g,b,block) run
        # run start: seg_off[g,b] + (blk%16)*tpb[g,b]*128; position = rank in run
        key = sblk * NBUCKET + sbkt
        # stable order already (blk, bkt); rank within run:
        runstart_mark = np.r_[True, key[1:] != key[:-1]]
        runid = np.cumsum(runstart_mark) - 1
        nruns = int(runid[-1]) + 1 if len(runid) else 0
        first = np.full(nruns, len(sel), np.int64)
        np.minimum.at(first, runid, np.arange(len(sel)))
        pos = np.arange(len(sel)) - first[runid]
        gg = sblk // 16
        slot = (seg_off[gg, sbkt] + (sblk % 16) * tpb[gg, sbkt] * 128 + pos)
        assert (pos < tpb[gg, sbkt] * 128).all()
        p_ = slot % 128
        t_ = slot // 128
        src_local[c, p_, t_] = sib.astype(np.int16)
        dst_loc[c, p_, t_] = (spd % NSLOT % 128)

    # static one-hot scatter tiles, fp8 (1.0/0.0 exact):
    # oh[p, t, c]  = (dst_loc[p, t] == c)   (edge on partition, dst on free)
    # ohT[c, t, e] = (dst_loc[e, t] == c)   (dst on partition, edge on free)
    ar = np.arange(128, dtype=np.int64)
    oh_all = np.zeros((NCORES, 128, TOTSLOT // 128, 128), fp8)
    ohT_all = np.zeros((NCORES, 128, TOTSLOT // 128, 128), fp8)
    for c in range(NCORES):
        ohb = dst_loc[c][:, :, None] == ar[None, None, :]  # [p, t, c]
        oh_all[c] = ohb.astype(fp8)
        ohT_all[c] = np.ascontiguousarray(ohb.transpose(2, 1, 0)).astype(fp8)

    # wrapped int16 gather indices per (g,b) call, concatenated along cols
    idx_w = np.zeros((NCORES, 128, TOTSLOT // 16), np.int16)
    for c in range(NCORES):
        flat = np.zeros(TOTSLOT, np.int16)
        sl = src_local[c]
        flat[np.arange(TOTSLOT)] = sl[np.arange(TOTSLOT) % 128,
                                      np.arange(TOTSLOT) // 128]
        w = flat.reshape(TOTSLOT // 16, 16).T  # [16, TOTSLOT//16]
        idx_w[c] = np.tile(w, (8, 1))

    # weights
    W1 = np.asarray(inputs["W1"], np.float32)
    W2 = np.asarray(inputs["W2"], np.float32)
    waug1 = np.concatenate(
        [W1, (W1 @ np.asarray(inputs["a_src1"], np.float32))[:, None],
         (W1 @ np.asarray(inputs["a_dst1"], np.float32))[:, None]], axis=1)
    waug2 = np.concatenate(
        [W2, (W2 @ np.asarray(inputs["a_src2"], np.float32))[:, None],
         (W2 @ np.asarray(inputs["a_dst2"], np.float32))[:, None]], axis=1)

    b1 = np.asarray(inputs["b1"], np.float32)
    b2v = np.asarray(inputs["b2"], np.float32)
    lin1_W = np.asarray(inputs["lin1_W"], np.float32)
    lin1_b = np.asarray(inputs["lin1_b"], np.float32)
    lin2_W = np.asarray(inputs["lin2_W"], np.float32)
    lin2_b = np.asarray(inputs["lin2_b"], np.float32)

    invcnt = np.zeros((NCORES, 64, GPC), np.float32)
    for c in range(NCORES):
        invcnt[c] = np.broadcast_to(
            1.0 / np.maximum(cnt[c * GPC:(c + 1) * GPC], 1.0), (64, GPC))

    tpl = dict(SLOT=SLOT, NSLOT=NSLOT, NB=NB, NGRP=NGRP, NSLOT_G=NSLOT_G,
               BUCKET=BUCKET, tpb=tpb, seg_tiles=seg_tiles, seg_off=seg_off,
               TT=TT, _src_local=src_local, _dst_loc=dst_loc, _x_pi=x_pi)

    per_core = []
    for c in range(NCORES):
        per_core.append({
            "xT_own": np.ascontiguousarray(x_pi[c].T),
            "idx_w": idx_w[c],
            "oh_hbm": oh_all[c].reshape(128, TOTSLOT),
            "ohT_hbm": ohT_all[c].reshape(128, TOTSLOT),
            "mask01": mask01[c],
            "maskpad": 1.0 - mask01[c],
            "invcnt": invcnt[c],
            "waug1": waug1,
            "waug2_bf": waug2.astype(bf16),
            "b1_tile": np.broadcast_to(b1, (128, 64)).copy(),
            "b1_tile2": np.broadcast_to(b2v, (128, 64)).copy(),
            "lin1_W": lin1_W,
            "lin1b_tile": np.broadcast_to(lin1_b, (128, 64)).copy(),
            "lin2row": np.broadcast_to(lin2_W[:, 0], (128, 64)).copy(),
            "b2col": np.full((128, 1), lin2_b[0], np.float32),
        })
    return tpl, per_core


IN_KEYS = ["xT_own", "idx_w", "oh_hbm", "ohT_hbm", "mask01", "maskpad",
           "invcnt", "waug1", "waug2_bf", "b1_tile", "b1_tile2", "lin1_W",
           "lin1b_tile", "lin2row", "b2col"]


# ---------------------------------------------------------------- device bld
DEBUG = False
SIM_NO_COLLECTIVE = False  # analysis only: single-core, collectives -> DMA


def _build(tpl):
    NSLOT = tpl["NSLOT"]
    NB = tpl["NB"]
    NGRP = tpl["NGRP"]
    NSLOT_G = tpl["NSLOT_G"]
    BUCKET = tpl["BUCKET"]
    tpb = tpl["tpb"]
    seg_tiles = tpl["seg_tiles"]
    TT = tpl["TT"]
    MAXNT = int(seg_tiles.max())

    nc = bacc.Bacc("TRN2", target_bir_lowering=False, debug=False,
                   num_devices=1 if SIM_NO_COLLECTIVE else NCORES)

    # inputs
    xT_own = nc.dram_tensor("xT_own", [FIN, NSLOT], F32,
                            kind="ExternalInput")
    idx_w = nc.dram_tensor("idx_w", [128, TT * 8], I16, kind="ExternalInput")
    oh_hbm = nc.dram_tensor("oh_hbm", [128, TT * 128], F8,
                            kind="ExternalInput")
    ohT_hbm = nc.dram_tensor("ohT_hbm", [128, TT * 128], F8,
                             kind="ExternalInput")
    mask01 = nc.dram_tensor("mask01", [128, NB], F32, kind="ExternalInput")
    maskpad = nc.dram_tensor("maskpad", [128, NB], F32, kind="ExternalInput")
    invcnt = nc.dram_tensor("invcnt", [64, GPC], F32, kind="ExternalInput")
    waug1 = nc.dram_tensor("waug1", [64, 66], F32, kind="ExternalInput")
    waug2_bf = nc.dram_tensor("waug2_bf", [64, 66], BF, kind="ExternalInput")
    b1_tile = nc.dram_tensor("b1_tile", [128, 64], F32, kind="ExternalInput")
    b1_tile2 = nc.dram_tensor("b1_tile2", [128, 64], F32, kind="ExternalInput")
    lin1_W = nc.dram_tensor("lin1_W", [128, 64], F32, kind="ExternalInput")
    lin1b_tile = nc.dram_tensor("lin1b_tile", [128, 64], F32,
                                kind="ExternalInput")
    lin2row = nc.dram_tensor("lin2row", [128, 64], F32, kind="ExternalInput")
    b2col = nc.dram_tensor("b2col", [128, 1], F32, kind="ExternalInput")

    out_final = nc.dram_tensor("out_final", [128, 1], F32,
                               kind="ExternalOutput")
    if DEBUG:
        dbg_tab = nc.dram_tensor("dbg_tab", [NSLOT, 65], BF,
                                 kind="ExternalOutput")
        dbg_d = nc.dram_tensor("dbg_d", [128, NB], F32, kind="ExternalOutput")
        dbg_hT = nc.dram_tensor("dbg_hT", [64, NSLOT], BF,
                                kind="ExternalOutput")
        dbg_hT2 = nc.dram_tensor("dbg_hT2", [64, NSLOT], BF,
                                 kind="ExternalOutput")
        dbg_pool = nc.dram_tensor("dbg_pool", [64, 4 * GPC], F32,
                                  kind="ExternalOutput")
        dbg_z = nc.dram_tensor("dbg_z", [G, 128], F32, kind="ExternalOutput")
        dbg_dps = nc.dram_tensor("dbg_dps", [128, 4 * MAXNT], F32,
                                 kind="ExternalOutput")
        dbg_scol = nc.dram_tensor("dbg_scol", [128, 4 * MAXNT], F32,
                                  kind="ExternalOutput")
        dbg_w = nc.dram_tensor("dbg_w", [128, 4 * MAXNT], F32,
                               kind="ExternalOutput")
        dbg_ps0 = nc.dram_tensor("dbg_ps0", [128, 260], F32,
                                 kind="ExternalOutput")

    with tile.TileContext(nc) as tc:
        with (
            tc.tile_pool(name="const", bufs=1) as cp,
            tc.tile_pool(name="stage", bufs=1) as stp,
            tc.tile_pool(name="dram", bufs=1, space="DRAM") as dr,
        ):
            # ---- constants in SBUF
            ident = cp.tile([128, 128], F32)
            make_identity(nc, ident[:])
            ones_bf = cp.tile([128, 1], BF)
            nc.gpsimd.memset(ones_bf[:], 1.0)
            zeros64 = cp.tile([128, 64], F32)
            nc.vector.memset(zeros64[:], 0.0)
            waug1_sb = cp.tile([64, 66], F32)
            nc.sync.dma_start(waug1_sb[:], waug1[:, :])
            waug2_sb = cp.tile([64, 66], BF)
            nc.sync.dma_start(waug2_sb[:], waug2_bf[:, :])
            b1t = cp.tile([128, 64], F32)
            nc.sync.dma_start(b1t[:], b1_tile[:, :])
            b2t = cp.tile([128, 64], F32)
            nc.sync.dma_start(b2t[:], b1_tile2[:, :])
            m01 = cp.tile([128, NB], F32)
            nc.sync.dma_start(m01[:], mask01[:, :])
            mpad = cp.tile([128, NB], F32)
            nc.sync.dma_start(mpad[:], maskpad[:, :])

            # persistent staging
            h_T = [stp.tile([64, NSLOT], BF, tag=f"h_T{l}", name=f"h_T{l}")
                   for l in range(2)]  # per-layer node features, pi order
            dstage = [stp.tile([128, NB], F32, tag=f"dstage{l}",
                               name=f"dstage{l}") for l in range(2)]
            dstage_bf = [stp.tile([128, NB], BF, tag=f"dstb{l}",
                                  name=f"dstb{l}") for l in range(2)]

            # DRAM scratch
            table = [dr.tile([NSLOT_G, 128], BF, tag=f"tab{l}", name=f"tab{l}")
                     for l in range(2)]
            tab_own = [dr.tile([NSLOT, 128], BF, tag=f"tabown{l}",
                               name=f"tabown{l}") for l in range(2)]
            pool_bounce_in = dr.tile([GPC, 128], F32)
            pool_bounce_out = dr.tile([G, 128], F32)

            # ===== table 1 build (own shard) =====
            with (
                tc.tile_pool(name="p1", bufs=3) as p1,
                tc.tile_pool(name="p1x", bufs=3) as p1x,
                tc.tile_pool(name="p1ps", bufs=2, space="PSUM") as p1ps,
            ):
                SLAB = 32
                for base in range(0, NB, SLAB):
                    ns = min(SLAB, NB - base)
                    xT = p1x.tile([64, SLAB * 128], F32, tag="xTs")
                    nc.sync.dma_start(
                        xT[:, 0:ns * 128],
                        xT_own[:, 128 * base:128 * (base + ns)])
                    rows = p1.tile([128, SLAB, 65], BF, tag="rows")
                    for j in range(ns):
                        hps = p1ps.tile([128, 66], F32, space="PSUM",
                                        tag="hps")
                        nc.tensor.matmul(
                            hps[:], lhsT=xT[:, 128 * j:128 * (j + 1)],
                            rhs=waug1_sb[:], start=True, stop=True)
                        nc.scalar.activation(rows[:, j, :], hps[:, 0:65],
                                             ACT.Copy)
                        nc.vector.tensor_copy(
                            dstage[0][:, base + j:base + j + 1],
                            hps[:, 65:66])
                    nc.sync.dma_start(
                        tab_own[0].rearrange("(s r) c -> r s c", r=128)[
                            :, base:base + ns, 0:65],
                        rows[:, 0:ns, :])
                nc.vector.tensor_copy(dstage_bf[0][:], dstage[0][:])

            QUARTER = NSLOT // NBUCKET

            def _table_allgather(l):
                # 4 quarter-AllGathers: bucket q of table[l] is complete as
                # soon as AG #q lands, so bucket-q gathers (and the whole
                # edge pipeline behind them) overlap the remaining AGs.
                for q in range(NBUCKET):
                    src_ap = tab_own[l][q * QUARTER:(q + 1) * QUARTER, :]
                    dst_ap = table[l][q * BUCKET:(q + 1) * BUCKET, :]
                    if SIM_NO_COLLECTIVE:
                        for i in range(NCORES):
                            nc.sync.dma_start(
                                table[l][q * BUCKET + i * QUARTER:
                                         q * BUCKET + (i + 1) * QUARTER, :],
                                src_ap)
                    else:
                        nc.gpsimd.collective_compute(
                            "AllGather", AL.bypass,
                            replica_groups=[list(range(NCORES))],
                            ins=[src_ap], outs=[dst_ap])

            _table_allgather(0)

            # ================= edge pass (both layers) =================
            def edge_pass(l):
                tab = table[l]
                with (
                    tc.tile_pool(name=f"eg{l}", bufs=2) as eg,
                    tc.tile_pool(name=f"eo{l}", bufs=2) as eo,
                    tc.tile_pool(name=f"eq{l}", bufs=2) as eqp,
                    tc.tile_pool(name=f"ep{l}", bufs=1, space="PSUM") as eps,
                    tc.tile_pool(name=f"ed{l}", bufs=2, space="PSUM") as edp,
                    tc.tile_pool(name=f"et{l}", bufs=1, space="PSUM") as ept,
                    tc.tile_pool(name=f"ef{l}", bufs=3) as ef,
                ):
                    for g in range(NGRP):
                        psums = [eps.tile([128, 260], F32, space="PSUM",
                                          tag=f"ps{k}", name=f"ps{k}")
                                 for k in range(4)]
                        for ps_ in psums:
                            nc.vector.memset(ps_[:], 0.0)

                        for b in range(NBUCKET):
                            ntile = int(seg_tiles[g, b])
                            Tb = int(tpb[g, b])
                            off = int(tpl["seg_off"][g, b]) // 128  # tile off
                            # gather the whole (g,b) segment
                            idx_sb = eg.tile([128, MAXNT * 8], I16, tag="idx")
                            nc.sync.dma_start(
                                idx_sb[:, 0:ntile * 8],
                                idx_w[:, off * 8:(off + ntile) * 8])
                            ch = eg.tile([128, MAXNT, 128], BF, tag="ch")
                            nc.gpsimd.dma_gather(
                                out_ap=ch[:, 0:ntile, :],
                                in_ap=tab[b * BUCKET:(b + 1) * BUCKET, :],
                                idxs_ap=idx_sb[:, 0:ntile * 8],
                                num_idxs=ntile * 128,
                                num_idxs_reg=ntile * 128,
                                elem_size=128,
                                single_packet=False)
                            # static one-hots for this segment
                            oh = eo.tile([128, MAXNT, 128], F8, tag="oh")
                            nc.sync.dma_start(
                                oh[:, 0:ntile, :],
                                oh_hbm[:, off * 128:(off + ntile) * 128])
                            ohT = eo.tile([128, MAXNT, 128], F8, tag="ohT")
                            nc.sync.dma_start(
                                ohT[:, 0:ntile, :],
                                ohT_hbm[:, off * 128:(off + ntile) * 128])
                            # d_dst per edge: one matvec per tile
                            dps = edp.tile([128, MAXNT], F32, space="PSUM",
                                           tag="dps")
                            for tt in range(ntile):
                                k16 = tt // Tb
                                nc.tensor.matmul(
                                    dps[:, tt:tt + 1],
                                    lhsT=ohT[:, tt, :],
                                    rhs=dstage_bf[l][:, 16 * g + k16:
                                                     16 * g + k16 + 1],
                                    start=True, stop=True)
                            # z = s + d ; w = exp(max(0.2 z, z))
                            z = ef.tile([128, MAXNT], F32, tag="z")
                            nc.vector.tensor_tensor(
                                out=z[:, 0:ntile], in0=ch[:, 0:ntile, 64],
                                in1=dps[:, 0:ntile], op=AL.add)
                            nc.vector.scalar_tensor_tensor(
                                out=z[:, 0:ntile], in0=z[:, 0:ntile],
                                scalar=NEG, in1=z[:, 0:ntile],
                                op0=AL.mult, op1=AL.max)
                            w = ef.tile([128, MAXNT], F32, tag="w")
                            nc.scalar.activation(w[:, 0:ntile], z[:, 0:ntile],
                                                 ACT.Exp)
                            if DEBUG and l == 0 and g == 0:
                                stg = ef.tile([128, MAXNT], F32, tag="dbgstg",
                                              name="stg")
                                nc.vector.tensor_copy(stg[:, 0:ntile],
                                                      dps[:, 0:ntile])
                                nc.sync.dma_start(
                                    dbg_dps[:, b * MAXNT:b * MAXNT + ntile],
                                    stg[:, 0:ntile])
                                stg2 = ef.tile([128, MAXNT], F32,
                                               tag="dbgstg2", name="stg2")
                                nc.vector.tensor_copy(stg2[:, 0:ntile],
                                                      ch[:, 0:ntile, 64])
                                nc.sync.dma_start(
                                    dbg_scol[:, b * MAXNT:b * MAXNT + ntile],
                                    stg2[:, 0:ntile])
                                nc.sync.dma_start(
                                    dbg_w[:, b * MAXNT:b * MAXNT + ntile],
                                    w[:, 0:ntile])
                            # ones into s slot for the denominator column
                            nc.vector.tensor_copy(
                                ch[:, 0:ntile, 64],
                                ones_bf[:].to_broadcast([128, ntile]))
                            # rhs = ch[:, :, 0:65] * w (bulk)
                            ch2 = eqp.tile([128, MAXNT, 65], BF, tag="ch2")
                            nc.vector.tensor_tensor(
                                out=ch2[:, 0:ntile, :], in0=ch[:, 0:ntile, 0:65],
                                in1=w[:, 0:ntile].to_broadcast(
                                    [128, ntile, 65]),
                                op=AL.mult)
                            # scatter matmuls
                            for i in range(16):
                                ps = psums[i // 4]
                                csl = slice(65 * (i % 4), 65 * (i % 4) + 65)
                                for t in range(Tb):
                                    tt = i * Tb + t
                                    nc.tensor.matmul(
                                        ps[:, csl],
                                        lhsT=oh[:, tt, :],
                                        rhs=ch2[:, tt, :],
                                        start=False,
                                        stop=(b == NBUCKET - 1 and t == Tb - 1))

                        if DEBUG and l == 0 and g == 0:
                            stg3 = ef.tile([128, 260], F32, tag="dbgstg3",
                                           name="stg3")
                            nc.vector.tensor_copy(stg3[:], psums[0][:])
                            nc.sync.dma_start(dbg_ps0[:, :], stg3[:])

                        # ---- finalize the group's 16 blocks (batched)
                        rows_g = ef.tile([128, 16, 65], BF, tag="rows_g")
                        nc.sync.dma_start(
                            rows_g[:],
                            tab_own[l].rearrange("(s r) c -> r s c", r=128)[
                                :, 16 * g:16 * (g + 1), 0:65])
                        gs = slice(16 * g, 16 * (g + 1))
                        zs = ef.tile([128, 16], F32, tag="zs")
                        nc.vector.tensor_tensor(
                            out=zs[:], in0=rows_g[:, :, 64],
                            in1=dstage[l][:, gs], op=AL.add)
                        nc.vector.scalar_tensor_tensor(
                            out=zs[:], in0=zs[:], scalar=NEG, in1=zs[:],
                            op0=AL.mult, op1=AL.max)
                        ws = ef.tile([128, 16], F32, tag="ws")
                        nc.scalar.activation(ws[:], zs[:], ACT.Exp)
                        nc.vector.tensor_tensor(
                            out=ws[:], in0=ws[:], in1=m01[:, gs], op=AL.mult)
                        nc.vector.tensor_copy(
                            rows_g[:, :, 64], ones_bf[:].to_broadcast([128, 16]))
                        for i in range(16):
                            ps = psums[i // 4]
                            csl = slice(65 * (i % 4), 65 * (i % 4) + 65)
                            nc.vector.scalar_tensor_tensor(
                                out=ps[:, csl], in0=rows_g[:, i, :],
                                scalar=ws[:, i:i + 1], in1=ps[:, csl],
                                op0=AL.mult, op1=AL.add)
                        # denominators -> reciprocal (batched)
                        den = ef.tile([128, 16], F32, tag="den")
                        for q in range(4):
                            nc.vector.tensor_copy(
                                den[:, 4 * q:4 * q + 4],
                                psums[q].rearrange("p (i c) -> p i c", i=4)[
                                    :, :, 64])
                        nc.vector.tensor_tensor(
                            out=den[:], in0=den[:], in1=mpad[:, gs], op=AL.add)
                        rec = ef.tile([128, 16], F32, tag="rec")
                        nc.vector.reciprocal(rec[:], den[:])
                        for i in range(16):
                            blk_id = 16 * g + i
                            ps = psums[i // 4]
                            c0 = 65 * (i % 4)
                            hmid = ef.tile([128, 64], F32, tag="hmid")
                            nc.vector.scalar_tensor_tensor(
                                out=hmid[:], in0=ps[:, c0:c0 + 64],
                                scalar=rec[:, i:i + 1],
                                in1=b1t[:] if l == 0 else b2t[:],
                                op0=AL.mult, op1=AL.add)
                            # relu + pad-mask in one op: max(mask*h, 0)
                            hout = ef.tile([128, 64], F32, tag="hout")
                            nc.vector.scalar_tensor_tensor(
                                out=hout[:], in0=hmid[:],
                                scalar=m01[:, blk_id:blk_id + 1],
                                in1=zeros64[:], op0=AL.mult, op1=AL.max)
                            # transpose (PE) -> [64, 128], store bf16
                            hT = ept.tile([64, 128], F32, space="PSUM",
                                          tag="hT")
                            nc.tensor.transpose(hT[:], hout[:], ident[:])
                            nc.vector.tensor_copy(
                                h_T[l][:, 128 * blk_id:128 * (blk_id + 1)],
                                hT[:])
                            if l == 0:
                                # fused layer-2 table row build
                                hps2 = ept.tile([128, 66], F32, space="PSUM",
                                                tag="hps2")
                                nc.tensor.matmul(
                                    hps2[:],
                                    lhsT=h_T[0][:, 128 * blk_id:
                                                128 * (blk_id + 1)],
                                    rhs=waug2_sb[:], start=True, stop=True)
                                rows2 = ef.tile([128, 65], BF, tag="rows2")
                                nc.vector.tensor_copy(rows2[:], hps2[:, 0:65])
                                nc.sync.dma_start(
                                    tab_own[1][128 * blk_id:
                                               128 * (blk_id + 1), 0:65],
                                    rows2[:])
                                nc.vector.tensor_copy(
                                    dstage[1][:, blk_id:blk_id + 1],
                                    hps2[:, 65:66])

            edge_pass(0)
            nc.vector.tensor_copy(dstage_bf[1][:], dstage[1][:])
            _table_allgather(1)
            edge_pass(1)

            if DEBUG:
                nc.sync.dma_start(dbg_tab[:, :], tab_own[0][:, 0:65])
                nc.sync.dma_start(dbg_d[:, :], dstage[0][:])
                nc.sync.dma_start(dbg_hT[:, :], h_T[0][:])
                nc.sync.dma_start(dbg_hT2[:, :], h_T[1][:])

            # ================= pooling + MLP =================
            with (
                tc.tile_pool(name="p5", bufs=2) as p5,
                tc.tile_pool(name="p5ps", bufs=1, space="PSUM") as p5ps,
            ):
                # pool over stored h_T: column c = 128*b + 8*pg + p8
                # view [64, pg=16, b=NB, p8=8], reduce XY (b, p8)
                mx = [p5.tile([64, GPC], F32, tag=f"mx{l}", name=f"mx{l}") for l in range(2)]
                sm = [p5.tile([64, GPC], F32, tag=f"sm{l}", name=f"sm{l}") for l in range(2)]
                for l in range(2):
                    v = h_T[l][:].rearrange("f (b pg p8) -> f pg b p8",
                                            b=NB, pg=GPC, p8=8)
                    nc.vector.tensor_reduce(
                        mx[l][:], v, axis=mybir.AxisListType.XY, op=AL.max)
                    nc.vector.tensor_reduce(
                        sm[l][:], v, axis=mybir.AxisListType.XY, op=AL.add)
                icn = p5.tile([64, GPC], F32)
                nc.sync.dma_start(icn[:], invcnt[:, :])
                mxh = p5.tile([64, GPC], F32)
                nc.vector.tensor_tensor(out=mxh[:], in0=mx[0][:], in1=mx[1][:],
                                        op=AL.add)
                smh = p5.tile([64, GPC], F32)
                nc.vector.tensor_tensor(out=smh[:], in0=sm[0][:], in1=sm[1][:],
                                        op=AL.add)
                nc.vector.tensor_tensor(out=smh[:], in0=smh[:], in1=icn[:],
                                        op=AL.mult)
                if DEBUG:
                    nc.sync.dma_start(dbg_pool[:, 0:GPC], mx[0][:])
                    nc.sync.dma_start(dbg_pool[:, GPC:2 * GPC], mx[1][:])
                    nc.sync.dma_start(dbg_pool[:, 2 * GPC:3 * GPC], sm[0][:])
                    nc.sync.dma_start(dbg_pool[:, 3 * GPC:4 * GPC], sm[1][:])
                # transpose to graph-major [GPC, 128] and AllGather
                zloc = p5.tile([GPC, 128], F32)
                mxT = p5ps.tile([GPC, 64], F32, space="PSUM", tag="mxT")
                nc.tensor.transpose(mxT[:], mxh[:], ident[0:64, 0:64])
                nc.vector.tensor_copy(zloc[:, 0:64], mxT[:])
                smT = p5ps.tile([GPC, 64], F32, space="PSUM", tag="smT")
                nc.tensor.transpose(smT[:], smh[:], ident[0:64, 0:64])
                nc.vector.tensor_copy(zloc[:, 64:128], smT[:])
                nc.sync.dma_start(pool_bounce_in[:, :], zloc[:])
                if SIM_NO_COLLECTIVE:
                    for i in range(NCORES):
                        nc.sync.dma_start(
                            pool_bounce_out[i * GPC:(i + 1) * GPC, :],
                            pool_bounce_in[:, :])
                else:
                    nc.gpsimd.collective_compute(
                        "AllGather", AL.bypass,
                        replica_groups=[list(range(NCORES))],
                        ins=[pool_bounce_in.opt()],
                        outs=[pool_bounce_out.opt()])
                zg = p5.tile([G, 128], F32)
                nc.sync.dma_start(zg[:], pool_bounce_out[:, :])
                if DEBUG:
                    nc.sync.dma_start(dbg_z[:, :], zg[:])
                zT_ps = p5ps.tile([128, G], F32, space="PSUM", tag="zT")
                nc.tensor.transpose(zT_ps[:], zg[:], ident[:])
                zT = p5.tile([128, G], F32)
                nc.vector.tensor_copy(zT[:], zT_ps[:])
                l1w = p5.tile([128, 64], F32)
                nc.sync.dma_start(l1w[:], lin1_W[:, :])
                mlp_ps = p5ps.tile([G, 64], F32, space="PSUM", tag="mlp")
                nc.tensor.matmul(mlp_ps[:], lhsT=zT[:], rhs=l1w[:],
                                 start=True, stop=True)
                l1b = p5.tile([128, 64], F32)
                nc.sync.dma_start(l1b[:], lin1b_tile[:, :])
                z1 = p5.tile([G, 64], F32)
                nc.vector.tensor_tensor(out=z1[:], in0=mlp_ps[:], in1=l1b[:],
                                        op=AL.add)
                nc.scalar.activation(z1[:], z1[:], ACT.Relu)
                l2r = p5.tile([128, 64], F32)
                nc.sync.dma_start(l2r[:], lin2row[:, :])
                z2 = p5.tile([G, 64], F32)
                nc.vector.tensor_tensor(out=z2[:], in0=z1[:], in1=l2r[:],
                                        op=AL.mult)
                ored = p5.tile([G, 1], F32)
                nc.vector.tensor_reduce(ored[:], z2[:],
                                        axis=mybir.AxisListType.X, op=AL.add)
                b2c = p5.tile([128, 1], F32)
                nc.sync.dma_start(b2c[:], b2col[:, :])
                nc.vector.tensor_tensor(out=ored[:], in0=ored[:], in1=b2c[:],
                                        op=AL.add)
                nc.sync.dma_start(out_final[:, :], ored[:])

    nc.compile()
    return nc


# ---------------------------------------------------------------- entry
def kernel(**inputs) -> np.ndarray:
    tpl, per_core = _host_prep(inputs)
    nc = _build(tpl)
    in_maps = [{k: per_core[c][k] for k in IN_KEYS} for c in range(NCORES)]
    res = bass_utils.run_bass_kernel_spmd(
        nc, in_maps, core_ids=list(range(NCORES)))
    out = np.asarray(res.results[0]["out_final"]).reshape(G)
    return out.astype(np.float32)


# revision 26
# speedup vs baseline: 435.5661x; 1.0803x over previous
"""GAT (2-layer, heads=1) + pooling + MLP on 8 Trainium2 NeuronCores.

Strategy:
- Nodes are mapped to per-graph padded slots (SLOT = align128(max graph size)),
  graphs are sharded 16-per-core, and within each core slots are striped so
  that dst-block i holds slots s with s % NB == i.  Every 128-slot block then
  contains exactly 8 slots of each of the core's 16 graphs (partition p ->
  local graph p//8), which makes pooling segment boundaries compile-time.
- Edge aggregation: per-edge rows [h|s] are fetched with dma_gather (int16
  indices -> 4 src buckets).  The edge->dst one-hot matrices are STATIC
  (graph structure), so they are precomputed on host in fp8 and streamed
  from HBM: ohT tiles give d_dst per edge via a tiny PE matvec against the
  block's d column; oh tiles drive the segment-softmax scatter matmul
  (one [128e x 128dst] x [128e x 65] matmul per 128-edge tile accumulating
  [sum(w*h) | sum(w)] into a per-block PSUM).
- Self-loop edges are applied densely at finalize (no gather); finalize is
  batched per 16-block group, and the layer-2 node table is built in the
  same pass.  Pooling (max+sum) is done at the end with two strided
  tensor_reduce ops over the stored [64, NSLOT] feature tables.
- Node tables are built sharded and AllGathered; pooled [G,2H] is assembled
  with a single small AllGather; the final MLP is replicated.
"""
import sys

sys.path.insert(0, "/opt/trn_rl_repo")

import numpy as np
import ml_dtypes

import concourse.bacc as bacc
import concourse.bass as bass
import concourse.mybir as mybir
import concourse.tile as tile
from concourse import bass_utils
from concourse.masks import make_identity

bf16 = ml_dtypes.bfloat16
fp8 = ml_dtypes.float8_e4m3
F32 = mybir.dt.float32
BF = mybir.dt.bfloat16
F8 = mybir.dt.float8e4
I16 = mybir.dt.int16
AL = mybir.AluOpType
ACT = mybir.ActivationFunctionType

NCORES = 8
G = 128
N = 100000
FIN = 64
H = 64
NEG = 0.2
NBUCKET = 4
GPC = G // NCORES  # graphs per core = 16
NEG_BIG = -1.0e30


def _dma_gather_partial(gp, out_ap, in_ap, idxs_ap, num_idxs, elem_size,
                        elem_step, queue_num=0):
    """dma_gather with transfer size < row stride (elem_size in elements,
    row stride elem_step*dtype_size must be a multiple of 256B).  Bypasses
    bass's blanket elem_size%256 assert, which only the transpose path
    needs; verified exact on HW for 132B transfers from 256B rows."""
    from concourse.bass import exact_div
    stride_bytes = elem_step * mybir.dt.size(in_ap.dtype)
    stride_bytes_256 = exact_div(stride_bytes, 256)
    _in_ap = gp.lower_ap_dma(in_ap, for_custom_bir_dma=True)
    _idxs_ap = gp.lower_ap(idxs_ap)
    _out_ap = gp.lower_ap(out_ap)
    return gp.add_instruction(
        mybir.InstDMAGatherAnt(
            name=gp.bass.get_next_instruction_name(),
            ins=[*_in_ap, _idxs_ap, gp.lower_val_access(gp.to_reg(num_idxs))],
            outs=[_out_ap],
            transpose=False,
            num_idxs=num_idxs,
            elem_size=elem_size,
            stride_bytes_256=stride_bytes_256,
            gen_mode=0,
            single_packet=False,
            queue_num=queue_num,
            sbuf_tokens_per_rank=0,
            sbuf_free_dim_per_rank=0,
            sbuf_free_dim_pad_per_rank=0,
            sbuf_byte_offset=0,
        ))


# ---------------------------------------------------------------- host prep
def _host_prep(inputs):
    x = np.asarray(inputs["x"], np.float32)
    ei = np.asarray(inputs["edge_index"]).astype(np.int64)
    bid = np.asarray(inputs["batch_ids"]).astype(np.int64)

    cnt = np.bincount(bid, minlength=G).astype(np.int64)
    SLOT = int(np.ceil(max(cnt.max(), 128) / 128) * 128)
    NSLOT = GPC * SLOT
    NB = NSLOT // 128
    assert NB % 16 == 0, NB
    NGRP = NB // 16
    NSLOT_G = NCORES * NSLOT
    BUCKET = NSLOT_G // NBUCKET
    assert BUCKET * NBUCKET == NSLOT_G and BUCKET <= 32768

    gstart = np.zeros(G + 1, np.int64)
    gstart[1:] = np.cumsum(cnt)
    rank = np.arange(N, dtype=np.int64) - gstart[bid]
    slot_of = bid * SLOT + rank  # graph-padded slot, 0..NSLOT_G

    def pi(s):
        c, sl = s // NSLOT, s % NSLOT
        return c * NSLOT, (sl % NB) * 128 + sl // NB

    core_base, loc = pi(slot_of)
    pi_of = core_base + loc  # global pi row of each node

    # permuted x per core (pad rows zero)
    x_pi = np.zeros((NCORES, NSLOT, FIN), np.float32)
    x_pi[pi_of // NSLOT, pi_of % NSLOT] = x

    # masks / counts per core
    # local slot (block i, partition p) <-> graph-slot p*NB + i
    pp, ii = np.meshgrid(np.arange(128), np.arange(NB), indexing="ij")
    gslot = pp * NB + ii  # [128, NB] graph-padded local slot
    within = gslot % SLOT  # rank within graph
    lg = gslot // SLOT  # local graph 0..15 (== pp//8)
    mask01 = np.zeros((NCORES, 128, NB), np.float32)
    for c in range(NCORES):
        real = within < cnt[c * GPC + lg]
        mask01[c] = real.astype(np.float32)

    # edges (no self loops in the gather path)
    # src-bucket layout: table rows ordered [quarter, core, loc%QUARTER] so
    # bucket q is complete as soon as the q-th quarter-AllGather lands.
    QUARTER = NSLOT // NBUCKET
    src, dst = ei[0], ei[1]
    ps = pi_of[src]
    pd = pi_of[dst]
    core = pd // NSLOT
    blk = (pd % NSLOT) // 128
    bkt = (ps % NSLOT) // QUARTER
    idx_in_bucket = (ps // NSLOT) * QUARTER + (ps % NSLOT) % QUARTER
    grp = blk // 16

    cnts = np.zeros((NCORES, NB, NBUCKET), np.int64)
    np.add.at(cnts, (core, blk, bkt), 1)
    # uniform tiles-per-block within each (group, bucket)
    tpb = np.zeros((NGRP, NBUCKET), np.int64)  # tiles per block
    for g in range(NGRP):
        for b in range(NBUCKET):
            m = cnts[:, 16 * g:16 * g + 16, b].max()
            tpb[g, b] = max(1, int(np.ceil(m / 128)))
    seg_tiles = (16 * tpb).astype(np.int64)  # tiles per (g,b) segment
    TT = int(seg_tiles.sum())  # total tiles per core per layer
    TOTSLOT = TT * 128

    # slot offsets: order (g, b, block-within-group, slot)
    seg_off = np.zeros((NGRP, NBUCKET), np.int64)
    acc = 0
    for g in range(NGRP):
        for b in range(NBUCKET):
            seg_off[g, b] = acc
            acc += seg_tiles[g, b] * 128

    order = np.lexsort((bkt, blk))  # edges sorted by (blk, bkt); core split below
    src_local = np.zeros((NCORES, 128, TOTSLOT // 128), np.int16)
    dst_loc = np.full((NCORES, 128, TOTSLOT // 128), -1, np.int64)
    for c in range(NCORES):
        sel = order[core[order] == c]
        sblk, sbkt = blk[sel], bkt[sel]
        sib, spd = idx_in_bucket[sel], pd[sel]
        # slot index for each edge: within its (g,b,block) run
        # run start: seg_off[g,b] + (blk%16)*tpb[g,b]*128; position = rank in run
        key = sblk * NBUCKET + sbkt
        # stable order already (blk, bkt); rank within run:
        runstart_mark = np.r_[True, key[1:] != key[:-1]]
        runid = np.cumsum(runstart_mark) - 1
        nruns = int(runid[-1]) + 1 if len(runid) else 0
        first = np.full(nruns, len(sel), np.int64)
        np.minimum.at(first, runid, np.arange(len(sel)))
        pos = np.arange(len(sel)) - first[runid]
        gg = sblk // 16
        slot = (seg_off[gg, sbkt] + (sblk % 16) * tpb[gg, sbkt] * 128 + pos)
        assert (pos < tpb[gg, sbkt] * 128).all()
        p_ = slot % 128
        t_ = slot // 128
        src_local[c, p_, t_] = sib.astype(np.int16)
        dst_loc[c, p_, t_] = (spd % NSLOT % 128)

    # static one-hot scatter tiles, fp8 (1.0/0.0 exact):
    # oh[p, t, c]  = (dst_loc[p, t] == c)   (edge on partition, dst on free)
    # ohT[c, t, e] = (dst_loc[e, t] == c)   (dst on partition, edge on free)
    ar = np.arange(128, dtype=np.int64)
    oh_all = np.zeros((NCORES, 128, TOTSLOT // 128, 128), fp8)
    ohT_all = np.zeros((NCORES, 128, TOTSLOT // 128, 128), fp8)
    for c in range(NCORES):
        ohb = dst_loc[c][:, :, None] == ar[None, None, :]  # [p, t, c]
        oh_all[c] = ohb.astype(fp8)
        ohT_all[c] = np.ascontiguousarray(ohb.transpose(2, 1, 0)).astype(fp8)

    # wrapped int16 gather indices per (g,b) call, concatenated along cols
    idx_w = np.zeros((NCORES, 128, TOTSLOT // 16), np.int16)
    for c in range(NCORES):
        flat = np.zeros(TOTSLOT, np.int16)
        sl = src_local[c]
        flat[np.arange(TOTSLOT)] = sl[np.arange(TOTSLOT) % 128,
                                      np.arange(TOTSLOT) // 128]
        w = flat.reshape(TOTSLOT // 16, 16).T  # [16, TOTSLOT//16]
        idx_w[c] = np.tile(w, (8, 1))

    # weights
    W1 = np.asarray(inputs["W1"], np.float32)
    W2 = np.asarray(inputs["W2"], np.float32)
    waug1 = np.concatenate(
        [W1, (W1 @ np.asarray(inputs["a_src1"], np.float32))[:, None],
         (W1 @ np.asarray(inputs["a_dst1"], np.float32))[:, None]], axis=1)
    waug2 = np.concatenate(
        [W2, (W2 @ np.asarray(inputs["a_src2"], np.float32))[:, None],
         (W2 @ np.asarray(inputs["a_dst2"], np.float32))[:, None]], axis=1)

    b1 = np.asarray(inputs["b1"], np.float32)
    b2v = np.asarray(inputs["b2"], np.float32)
    lin1_W = np.asarray(inputs["lin1_W"], np.float32)
    lin1_b = np.asarray(inputs["lin1_b"], np.float32)
    lin2_W = np.asarray(inputs["lin2_W"], np.float32)
    lin2_b = np.asarray(inputs["lin2_b"], np.float32)

    invcnt = np.zeros((NCORES, 64, GPC), np.float32)
    for c in range(NCORES):
        invcnt[c] = np.broadcast_to(
            1.0 / np.maximum(cnt[c * GPC:(c + 1) * GPC], 1.0), (64, GPC))

    tpl = dict(SLOT=SLOT, NSLOT=NSLOT, NB=NB, NGRP=NGRP, NSLOT_G=NSLOT_G,
               BUCKET=BUCKET, tpb=tpb, seg_tiles=seg_tiles, seg_off=seg_off,
               TT=TT, _src_local=src_local, _dst_loc=dst_loc, _x_pi=x_pi)

    per_core = []
    for c in range(NCORES):
        per_core.append({
            "xT_own": np.ascontiguousarray(x_pi[c].T),
            "idx_w": idx_w[c],
            "oh_hbm": oh_all[c].reshape(128, TOTSLOT),
            "ohT_hbm": ohT_all[c].reshape(128, TOTSLOT),
            "mask01": mask01[c],
            "maskpad": 1.0 - mask01[c],
            "invcnt": invcnt[c],
            "waug1": waug1,
            "waug2_bf": waug2.astype(bf16),
            "b1_tile": np.broadcast_to(b1, (128, 64)).copy(),
            "b1_tile2": np.broadcast_to(b2v, (128, 64)).copy(),
            "lin1_W": lin1_W,
            "lin1b_tile": np.broadcast_to(lin1_b, (128, 64)).copy(),
            "lin2row": np.broadcast_to(lin2_W[:, 0], (128, 64)).copy(),
            "b2col": np.full((128, 1), lin2_b[0], np.float32),
        })
    return tpl, per_core


IN_KEYS = ["xT_own", "idx_w", "oh_hbm", "ohT_hbm", "mask01", "maskpad",
           "invcnt", "waug1", "waug2_bf", "b1_tile", "b1_tile2", "lin1_W",
           "lin1b_tile", "lin2row", "b2col"]


# ---------------------------------------------------------------- device bld
DEBUG = False
SIM_NO_COLLECTIVE = False  # analysis only: single-core, collectives -> DMA
ABLATE = frozenset()  # analysis only: {"gather", "onehot", "collective"}


def _build(tpl):
    NSLOT = tpl["NSLOT"]
    NB = tpl["NB"]
    NGRP = tpl["NGRP"]
    NSLOT_G = tpl["NSLOT_G"]
    BUCKET = tpl["BUCKET"]
    tpb = tpl["tpb"]
    seg_tiles = tpl["seg_tiles"]
    TT = tpl["TT"]
    MAXNT = int(seg_tiles.max())

    nc = bacc.Bacc("TRN2", target_bir_lowering=False, debug=False,
                   num_devices=1 if SIM_NO_COLLECTIVE else NCORES)

    # inputs
    xT_own = nc.dram_tensor("xT_own", [FIN, NSLOT], F32,
                            kind="ExternalInput")
    idx_w = nc.dram_tensor("idx_w", [128, TT * 8], I16, kind="ExternalInput")
    oh_hbm = nc.dram_tensor("oh_hbm", [128, TT * 128], F8,
                            kind="ExternalInput")
    ohT_hbm = nc.dram_tensor("ohT_hbm", [128, TT * 128], F8,
                             kind="ExternalInput")
    mask01 = nc.dram_tensor("mask01", [128, NB], F32, kind="ExternalInput")
    maskpad = nc.dram_tensor("maskpad", [128, NB], F32, kind="ExternalInput")
    invcnt = nc.dram_tensor("invcnt", [64, GPC], F32, kind="ExternalInput")
    waug1 = nc.dram_tensor("waug1", [64, 66], F32, kind="ExternalInput")
    waug2_bf = nc.dram_tensor("waug2_bf", [64, 66], BF, kind="ExternalInput")
    b1_tile = nc.dram_tensor("b1_tile", [128, 64], F32, kind="ExternalInput")
    b1_tile2 = nc.dram_tensor("b1_tile2", [128, 64], F32, kind="ExternalInput")
    lin1_W = nc.dram_tensor("lin1_W", [128, 64], F32, kind="ExternalInput")
    lin1b_tile = nc.dram_tensor("lin1b_tile", [128, 64], F32,
                                kind="ExternalInput")
    lin2row = nc.dram_tensor("lin2row", [128, 64], F32, kind="ExternalInput")
    b2col = nc.dram_tensor("b2col", [128, 1], F32, kind="ExternalInput")

    out_final = nc.dram_tensor("out_final", [128, 1], F32,
                               kind="ExternalOutput")
    if DEBUG:
        dbg_tab = nc.dram_tensor("dbg_tab", [NSLOT, 65], BF,
                                 kind="ExternalOutput")
        dbg_d = nc.dram_tensor("dbg_d", [128, NB], F32, kind="ExternalOutput")
        dbg_hT = nc.dram_tensor("dbg_hT", [64, NSLOT], BF,
                                kind="ExternalOutput")
        dbg_hT2 = nc.dram_tensor("dbg_hT2", [64, NSLOT], BF,
                                 kind="ExternalOutput")
        dbg_pool = nc.dram_tensor("dbg_pool", [64, 4 * GPC], F32,
                                  kind="ExternalOutput")
        dbg_z = nc.dram_tensor("dbg_z", [G, 128], F32, kind="ExternalOutput")
        dbg_dps = nc.dram_tensor("dbg_dps", [128, 4 * MAXNT], F32,
                                 kind="ExternalOutput")
        dbg_scol = nc.dram_tensor("dbg_scol", [128, 4 * MAXNT], F32,
                                  kind="ExternalOutput")
        dbg_w = nc.dram_tensor("dbg_w", [128, 4 * MAXNT], F32,
                               kind="ExternalOutput")
        dbg_ps0 = nc.dram_tensor("dbg_ps0", [128, 260], F32,
                                 kind="ExternalOutput")

    with tile.TileContext(nc) as tc:
        with (
            tc.tile_pool(name="const", bufs=1) as cp,
            tc.tile_pool(name="stage", bufs=1) as stp,
            tc.tile_pool(name="dram", bufs=1, space="DRAM") as dr,
        ):
            # ---- constants in SBUF
            ident = cp.tile([128, 128], F32)
            make_identity(nc, ident[:])
            ones_bf = cp.tile([128, 1], BF)
            nc.gpsimd.memset(ones_bf[:], 1.0)
            zeros64 = cp.tile([128, 64], F32)
            nc.vector.memset(zeros64[:], 0.0)
            waug1_sb = cp.tile([64, 66], F32)
            nc.sync.dma_start(waug1_sb[:], waug1[:, :])
            waug2_sb = cp.tile([64, 66], BF)
            nc.sync.dma_start(waug2_sb[:], waug2_bf[:, :])
            b1t = cp.tile([128, 64], F32)
            nc.sync.dma_start(b1t[:], b1_tile[:, :])
            b2t = cp.tile([128, 64], F32)
            nc.sync.dma_start(b2t[:], b1_tile2[:, :])
            m01 = cp.tile([128, NB], F32)
            nc.sync.dma_start(m01[:], mask01[:, :])
            mpad = cp.tile([128, NB], F32)
            nc.sync.dma_start(mpad[:], maskpad[:, :])

            # persistent staging
            h_T = [stp.tile([64, NSLOT], BF, tag=f"h_T{l}", name=f"h_T{l}")
                   for l in range(2)]  # per-layer node features, pi order
            dstage = [stp.tile([128, NB], F32, tag=f"dstage{l}",
                               name=f"dstage{l}") for l in range(2)]
            dstage_bf = [stp.tile([128, NB], BF, tag=f"dstb{l}",
                                  name=f"dstb{l}") for l in range(2)]

            # DRAM scratch
            table = [dr.tile([NSLOT_G, 128], BF, tag=f"tab{l}", name=f"tab{l}")
                     for l in range(2)]
            tab_own = [dr.tile([NSLOT, 128], BF, tag=f"tabown{l}",
                               name=f"tabown{l}") for l in range(2)]
            pool_bounce_in = dr.tile([GPC, 128], F32)
            pool_bounce_out = dr.tile([G, 128], F32)

            # ===== table 1 build (own shard) =====
            with (
                tc.tile_pool(name="p1", bufs=3) as p1,
                tc.tile_pool(name="p1x", bufs=3) as p1x,
                tc.tile_pool(name="p1ps", bufs=2, space="PSUM") as p1ps,
            ):
                SLAB = 32
                for base in range(0, NB, SLAB):
                    ns = min(SLAB, NB - base)
                    xT = p1x.tile([64, SLAB * 128], F32, tag="xTs")
                    nc.sync.dma_start(
                        xT[:, 0:ns * 128],
                        xT_own[:, 128 * base:128 * (base + ns)])
                    rows = p1.tile([128, SLAB, 65], BF, tag="rows")
                    for j in range(ns):
                        hps = p1ps.tile([128, 66], F32, space="PSUM",
                                        tag="hps")
                        nc.tensor.matmul(
                            hps[:], lhsT=xT[:, 128 * j:128 * (j + 1)],
                            rhs=waug1_sb[:], start=True, stop=True)
                        nc.scalar.activation(rows[:, j, :], hps[:, 0:65],
                                             ACT.Copy)
                        nc.vector.tensor_copy(
                            dstage[0][:, base + j:base + j + 1],
                            hps[:, 65:66])
                    nc.sync.dma_start(
                        tab_own[0].rearrange("(s r) c -> r s c", r=128)[
                            :, base:base + ns, 0:65],
                        rows[:, 0:ns, :])
                nc.vector.tensor_copy(dstage_bf[0][:], dstage[0][:])

            QUARTER = NSLOT // NBUCKET

            def _table_allgather(l):
                # 4 quarter-AllGathers: bucket q of table[l] is complete as
                # soon as AG #q lands, so bucket-q gathers (and the whole
                # edge pipeline behind them) overlap the remaining AGs.
                for q in range(NBUCKET):
                    src_ap = tab_own[l][q * QUARTER:(q + 1) * QUARTER, :]
                    dst_ap = table[l][q * BUCKET:(q + 1) * BUCKET, :]
                    if "collective" in ABLATE:
                        nc.sync.dma_start(
                            table[l][q * BUCKET:q * BUCKET + QUARTER, :],
                            src_ap)
                    elif SIM_NO_COLLECTIVE:
                        for i in range(NCORES):
                            nc.sync.dma_start(
                                table[l][q * BUCKET + i * QUARTER:
                                         q * BUCKET + (i + 1) * QUARTER, :],
                                src_ap)
                    else:
                        nc.gpsimd.collective_compute(
                            "AllGather", AL.bypass,
                            replica_groups=[list(range(NCORES))],
                            ins=[src_ap], outs=[dst_ap])

            _table_allgather(0)

            # ================= edge pass (both layers) =================
            def edge_pass(l):
                tab = table[l]
                with (
                    tc.tile_pool(name=f"eg{l}", bufs=2) as eg,
                    tc.tile_pool(name=f"eo{l}", bufs=2) as eo,
                    tc.tile_pool(name=f"eq{l}", bufs=2) as eqp,
                    tc.tile_pool(name=f"ep{l}", bufs=1, space="PSUM") as eps,
                    tc.tile_pool(name=f"ed{l}", bufs=2, space="PSUM") as edp,
                    tc.tile_pool(name=f"et{l}", bufs=1, space="PSUM") as ept,
                    tc.tile_pool(name=f"ef{l}", bufs=3) as ef,
                ):
                    for g in range(NGRP):
                        psums = [eps.tile([128, 260], F32, space="PSUM",
                                          tag=f"ps{k}", name=f"ps{k}")
                                 for k in range(4)]
                        for ps_ in psums:
                            nc.vector.memset(ps_[:], 0.0)

                        for b in range(NBUCKET):
                            ntile = int(seg_tiles[g, b])
                            Tb = int(tpb[g, b])
                            off = int(tpl["seg_off"][g, b]) // 128  # tile off
                            # gather the whole (g,b) segment
                            idx_sb = eg.tile([128, MAXNT * 8], I16, tag="idx")
                            nc.sync.dma_start(
                                idx_sb[:, 0:ntile * 8],
                                idx_w[:, off * 8:(off + ntile) * 8])
                            ch = eg.tile([128, MAXNT, 66], BF, tag="ch")
                            if "gather" not in ABLATE:
                                _dma_gather_partial(
                                    nc.gpsimd,
                                    out_ap=ch[:, 0:ntile, :],
                                    in_ap=tab[b * BUCKET:(b + 1) * BUCKET,
                                              0:66],
                                    idxs_ap=idx_sb[:, 0:ntile * 8],
                                    num_idxs=ntile * 128,
                                    elem_size=66,
                                    elem_step=128)
                            # static one-hots for this segment
                            oh = eo.tile([128, MAXNT, 128], F8, tag="oh")
                            ohT = eo.tile([128, MAXNT, 128], F8, tag="ohT")
                            if "onehot" not in ABLATE:
                                nc.sync.dma_start(
                                    oh[:, 0:ntile, :],
                                    oh_hbm[:, off * 128:(off + ntile) * 128])
                                nc.sync.dma_start(
                                    ohT[:, 0:ntile, :],
                                    ohT_hbm[:, off * 128:(off + ntile) * 128])
                            # d_dst per edge: one matvec per tile
                            dps = edp.tile([128, MAXNT], F32, space="PSUM",
                                           tag="dps")
                            for tt in range(ntile):
                                k16 = tt // Tb
                                nc.tensor.matmul(
                                    dps[:, tt:tt + 1],
                                    lhsT=ohT[:, tt, :],
                                    rhs=dstage_bf[l][:, 16 * g + k16:
                                                     16 * g + k16 + 1],
                                    start=True, stop=True)
                            # z = s + d ; w = exp(max(0.2 z, z))
                            z = ef.tile([128, MAXNT], F32, tag="z")
                            nc.vector.tensor_tensor(
                                out=z[:, 0:ntile], in0=ch[:, 0:ntile, 64],
                                in1=dps[:, 0:ntile], op=AL.add)
                            nc.vector.scalar_tensor_tensor(
                                out=z[:, 0:ntile], in0=z[:, 0:ntile],
                                scalar=NEG, in1=z[:, 0:ntile],
                                op0=AL.mult, op1=AL.max)
                            w = ef.tile([128, MAXNT], F32, tag="w")
                            nc.scalar.activation(w[:, 0:ntile], z[:, 0:ntile],
                                                 ACT.Exp)
                            if DEBUG and l == 0 and g == 0:
                                stg = ef.tile([128, MAXNT], F32, tag="dbgstg",
                                              name="stg")
                                nc.vector.tensor_copy(stg[:, 0:ntile],
                                                      dps[:, 0:ntile])
                                nc.sync.dma_start(
                                    dbg_dps[:, b * MAXNT:b * MAXNT + ntile],
                                    stg[:, 0:ntile])
                                stg2 = ef.tile([128, MAXNT], F32,
                                               tag="dbgstg2", name="stg2")
                                nc.vector.tensor_copy(stg2[:, 0:ntile],
                                                      ch[:, 0:ntile, 64])
                                nc.sync.dma_start(
                                    dbg_scol[:, b * MAXNT:b * MAXNT + ntile],
                                    stg2[:, 0:ntile])
                                nc.sync.dma_start(
                                    dbg_w[:, b * MAXNT:b * MAXNT + ntile],
                                    w[:, 0:ntile])
                            # ones into s slot for the denominator column
                            nc.vector.tensor_copy(
                                ch[:, 0:ntile, 64],
                                ones_bf[:].to_broadcast([128, ntile]))
                            # rhs = ch[:, :, 0:65] * w (bulk)
                            ch2 = eqp.tile([128, MAXNT, 65], BF, tag="ch2")
                            nc.vector.tensor_tensor(
                                out=ch2[:, 0:ntile, :], in0=ch[:, 0:ntile, 0:65],
                                in1=w[:, 0:ntile].to_broadcast(
                                    [128, ntile, 65]),
                                op=AL.mult)
                            # scatter matmuls
                            for i in range(16):
                                ps = psums[i // 4]
                                csl = slice(65 * (i % 4), 65 * (i % 4) + 65)
                                for t in range(Tb):
                                    tt = i * Tb + t
                                    nc.tensor.matmul(
                                        ps[:, csl],
                                        lhsT=oh[:, tt, :],
                                        rhs=ch2[:, tt, :],
                                        start=False,
                                        stop=(b == NBUCKET - 1 and t == Tb - 1))

                        if DEBUG and l == 0 and g == 0:
                            stg3 = ef.tile([128, 260], F32, tag="dbgstg3",
                                           name="stg3")
                            nc.vector.tensor_copy(stg3[:], psums[0][:])
                            nc.sync.dma_start(dbg_ps0[:, :], stg3[:])

                        # ---- finalize the group's 16 blocks (batched)
                        rows_g = ef.tile([128, 16, 65], BF, tag="rows_g")
                        nc.sync.dma_start(
                            rows_g[:],
                            tab_own[l].rearrange("(s r) c -> r s c", r=128)[
                                :, 16 * g:16 * (g + 1), 0:65])
                        gs = slice(16 * g, 16 * (g + 1))
                        zs = ef.tile([128, 16], F32, tag="zs")
                        nc.vector.tensor_tensor(
                            out=zs[:], in0=rows_g[:, :, 64],
                            in1=dstage[l][:, gs], op=AL.add)
                        nc.vector.scalar_tensor_tensor(
                            out=zs[:], in0=zs[:], scalar=NEG, in1=zs[:],
                            op0=AL.mult, op1=AL.max)
                        ws = ef.tile([128, 16], F32, tag="ws")
                        nc.scalar.activation(ws[:], zs[:], ACT.Exp)
                        nc.vector.tensor_tensor(
                            out=ws[:], in0=ws[:], in1=m01[:, gs], op=AL.mult)
                        nc.vector.tensor_copy(
                            rows_g[:, :, 64], ones_bf[:].to_broadcast([128, 16]))
                        for i in range(16):
                            ps = psums[i // 4]
                            csl = slice(65 * (i % 4), 65 * (i % 4) + 65)
                            nc.vector.scalar_tensor_tensor(
                                out=ps[:, csl], in0=rows_g[:, i, :],
                                scalar=ws[:, i:i + 1], in1=ps[:, csl],
                                op0=AL.mult, op1=AL.add)
                        # denominators -> reciprocal (batched)
                        den = ef.tile([128, 16], F32, tag="den")
                        for q in range(4):
                            nc.vector.tensor_copy(
                                den[:, 4 * q:4 * q + 4],
                                psums[q].rearrange("p (i c) -> p i c", i=4)[
                                    :, :, 64])
                        nc.vector.tensor_tensor(
                            out=den[:], in0=den[:], in1=mpad[:, gs], op=AL.add)
                        rec = ef.tile([128, 16], F32, tag="rec")
                        nc.vector.reciprocal(rec[:], den[:])
                        for i in range(16):
                            blk_id = 16 * g + i
                            ps = psums[i // 4]
                            c0 = 65 * (i % 4)
                            hmid = ef.tile([128, 64], F32, tag="hmid")
                            nc.vector.scalar_tensor_tensor(
                                out=hmid[:], in0=ps[:, c0:c0 + 64],
                                scalar=rec[:, i:i + 1],
                                in1=b1t[:] if l == 0 else b2t[:],
                                op0=AL.mult, op1=AL.add)
                            # relu + pad-mask in one op: max(mask*h, 0)
                            hout = ef.tile([128, 64], F32, tag="hout")
                            nc.vector.scalar_tensor_tensor(
                                out=hout[:], in0=hmid[:],
                                scalar=m01[:, blk_id:blk_id + 1],
                                in1=zeros64[:], op0=AL.mult, op1=AL.max)
                            # transpose (PE) -> [64, 128], store bf16
                            hT = ept.tile([64, 128], F32, space="PSUM",
                                          tag="hT")
                            nc.tensor.transpose(hT[:], hout[:], ident[:])
                            nc.vector.tensor_copy(
                                h_T[l][:, 128 * blk_id:128 * (blk_id + 1)],
                                hT[:])
                            if l == 0:
                                # fused layer-2 table row build
                                hps2 = ept.tile([128, 66], F32, space="PSUM",
                                                tag="hps2")
                                nc.tensor.matmul(
                                    hps2[:],
                                    lhsT=h_T[0][:, 128 * blk_id:
                                                128 * (blk_id + 1)],
                                    rhs=waug2_sb[:], start=True, stop=True)
                                rows2 = ef.tile([128, 65], BF, tag="rows2")
                                nc.vector.tensor_copy(rows2[:], hps2[:, 0:65])
                                nc.sync.dma_start(
                                    tab_own[1][128 * blk_id:
                                               128 * (blk_id + 1), 0:65],
                                    rows2[:])
                                nc.vector.tensor_copy(
                                    dstage[1][:, blk_id:blk_id + 1],
                                    hps2[:, 65:66])

            edge_pass(0)
            nc.vector.tensor_copy(dstage_bf[1][:], dstage[1][:])
            _table_allgather(1)
            edge_pass(1)

            if DEBUG:
                nc.sync.dma_start(dbg_tab[:, :], tab_own[0][:, 0:65])
                nc.sync.dma_start(dbg_d[:, :], dstage[0][:])
                nc.sync.dma_start(dbg_hT[:, :], h_T[0][:])
                nc.sync.dma_start(dbg_hT2[:, :], h_T[1][:])

            # ================= pooling + MLP =================
            with (
                tc.tile_pool(name="p5", bufs=2) as p5,
                tc.tile_pool(name="p5ps", bufs=1, space="PSUM") as p5ps,
            ):
                # pool over stored h_T: column c = 128*b + 8*pg + p8
                # view [64, pg=16, b=NB, p8=8], reduce XY (b, p8)
                mx = [p5.tile([64, GPC], F32, tag=f"mx{l}", name=f"mx{l}") for l in range(2)]
                sm = [p5.tile([64, GPC], F32, tag=f"sm{l}", name=f"sm{l}") for l in range(2)]
                for l in range(2):
                    v = h_T[l][:].rearrange("f (b pg p8) -> f pg b p8",
                                            b=NB, pg=GPC, p8=8)
                    nc.vector.tensor_reduce(
                        mx[l][:], v, axis=mybir.AxisListType.XY, op=AL.max)
                    nc.vector.tensor_reduce(
                        sm[l][:], v, axis=mybir.AxisListType.XY, op=AL.add)
                icn = p5.tile([64, GPC], F32)
                nc.sync.dma_start(icn[:], invcnt[:, :])
                mxh = p5.tile([64, GPC], F32)
                nc.vector.tensor_tensor(out=mxh[:], in0=mx[0][:], in1=mx[1][:],
                                        op=AL.add)
                smh = p5.tile([64, GPC], F32)
                nc.vector.tensor_tensor(out=smh[:], in0=sm[0][:], in1=sm[1][:],
                                        op=AL.add)
                nc.vector.tensor_tensor(out=smh[:], in0=smh[:], in1=icn[:],
                                        op=AL.mult)
                if DEBUG:
                    nc.sync.dma_start(dbg_pool[:, 0:GPC], mx[0][:])
                    nc.sync.dma_start(dbg_pool[:, GPC:2 * GPC], mx[1][:])
                    nc.sync.dma_start(dbg_pool[:, 2 * GPC:3 * GPC], sm[0][:])
                    nc.sync.dma_start(dbg_pool[:, 3 * GPC:4 * GPC], sm[1][:])
                # transpose to graph-major [GPC, 128] and AllGather
                zloc = p5.tile([GPC, 128], F32)
                mxT = p5ps.tile([GPC, 64], F32, space="PSUM", tag="mxT")
                nc.tensor.transpose(mxT[:], mxh[:], ident[0:64, 0:64])
                nc.vector.tensor_copy(zloc[:, 0:64], mxT[:])
                smT = p5ps.tile([GPC, 64], F32, space="PSUM", tag="smT")
                nc.tensor.transpose(smT[:], smh[:], ident[0:64, 0:64])
                nc.vector.tensor_copy(zloc[:, 64:128], smT[:])
                nc.sync.dma_start(pool_bounce_in[:, :], zloc[:])
                if SIM_NO_COLLECTIVE:
                    for i in range(NCORES):
                        nc.sync.dma_start(
                            pool_bounce_out[i * GPC:(i + 1) * GPC, :],
                            pool_bounce_in[:, :])
                else:
                    nc.gpsimd.collective_compute(
                        "AllGather", AL.bypass,
                        replica_groups=[list(range(NCORES))],
                        ins=[pool_bounce_in.opt()],
                        outs=[pool_bounce_out.opt()])
                zg = p5.tile([G, 128], F32)
                nc.sync.dma_start(zg[:], pool_bounce_out[:, :])
                if DEBUG:
                    nc.sync.dma_start(dbg_z[:, :], zg[:])
                zT_ps = p5ps.tile([128, G], F32, space="PSUM", tag="zT")
                nc.tensor.transpose(zT_ps[:], zg[:], ident[:])
                zT = p5.tile([128, G], F32)
                nc.vector.tensor_copy(zT[:], zT_ps[:])
                l1w = p5.tile([128, 64], F32)
                nc.sync.dma_start(l1w[:], lin1_W[:, :])
                mlp_ps = p5ps.tile([G, 64], F32, space="PSUM", tag="mlp")
                nc.tensor.matmul(mlp_ps[:], lhsT=zT[:], rhs=l1w[:],
                                 start=True, stop=True)
                l1b = p5.tile([128, 64], F32)
                nc.sync.dma_start(l1b[:], lin1b_tile[:, :])
                z1 = p5.tile([G, 64], F32)
                nc.vector.tensor_tensor(out=z1[:], in0=mlp_ps[:], in1=l1b[:],
                                        op=AL.add)
                nc.scalar.activation(z1[:], z1[:], ACT.Relu)
                l2r = p5.tile([128, 64], F32)
                nc.sync.dma_start(l2r[:], lin2row[:, :])
                z2 = p5.tile([G, 64], F32)
                nc.vector.tensor_tensor(out=z2[:], in0=z1[:], in1=l2r[:],
                                        op=AL.mult)
                ored = p5.tile([G, 1], F32)
                nc.vector.tensor_reduce(ored[:], z2[:],
                                        axis=mybir.AxisListType.X, op=AL.add)
                b2c = p5.tile([128, 1], F32)
                nc.sync.dma_start(b2c[:], b2col[:, :])
                nc.vector.tensor_tensor(out=ored[:], in0=ored[:], in1=b2c[:],
                                        op=AL.add)
                nc.sync.dma_start(out_final[:, :], ored[:])

    nc.compile()
    return nc


# ---------------------------------------------------------------- entry
def kernel(**inputs) -> np.ndarray:
    tpl, per_core = _host_prep(inputs)
    nc = _build(tpl)
    in_maps = [{k: per_core[c][k] for k in IN_KEYS} for c in range(NCORES)]
    res = bass_utils.run_bass_kernel_spmd(
        nc, in_maps, core_ids=list(range(NCORES)))
    out = np.asarray(res.results[0]["out_final"]).reshape(G)
    return out.astype(np.float32)
